# revision 18
# baseline (speedup 1.0000x reference)
"""Conv1d-QKV + full attention kernel for TRN2, 8 NeuronCores.

Problem (hardcoded shapes): B=4, S=4096, DIN=DQ=DK=256.
  q = conv1d(query, q_w, q_b); k = conv1d(key, ...); v = conv1d(value, ...)
  out = scale * softmax(q @ k^T / sqrt(256)) @ v

Sharding: 8 cores = (batch b = core//2) x (query half h = core%2).
Each core computes k/v conv over the full 4096 rows of its batch
(replicated across the 2 cores of a batch) and attention for its 2048
query rows.

All matmul speed comes from fp8e4m3/e5m2 DoubleRow at 0.5 cycles/row
(contracts 2x128 per instruction, 4x the f32r rate):
  - convs: hi/lo fp8 split of inputs and (64x prescaled) weights on the
    host; 3-term product xh*wh + xh*wl + xl*wh per tap (~0.15% error).
  - scores mm: q,k cast to fp8e4m3 (in 64x units) by the conv bias-add
    on DVE; DoubleRow over ci chunks. Dominant error term (~1e-2
    scale-relative; gate is 2e-2).
  - softmax weights are CENTERED before quantization: the activation
    computes expm1(s/16) via a patched act table (exp with 1.0
    subtracted from every d0 Taylor coefficient), so fp8's ~2.7%
    relative noise applies to e-1 (std ~0.37) instead of e (~1.1) --
    3x less noise. out = scale*(Sv + e_dev^T v)/(4096 + sum(e_dev)),
    where Sv = column sums of v from one extra DoubleRow chain.
  - out mm: v split into vh (e4m3) + vl (e5m2 residual); both fp8
    DoubleRow passes against e_dev. The Pool engine does the split.
Scale folds are exact powers of two: no extra ops, no extra error.
"""

import os
import json
import shutil
import struct

import numpy as np
import ml_dtypes

B, S, DIN, D = 4, 4096, 256, 256
NCORES = 8
SQ = S // 2          # query rows per core
SK = S               # key rows per core
DCH = 2              # 128-partition chunks of D / DIN
CT = 512             # conv column tile
QB = 512             # query block for attention
NBLK = SQ // QB      # 4
NKT = SK // 128      # 32 k tiles
DV = D + 4           # v_aug cols: D of v + [1,0,0,0] denominator col
VROW = 272           # v row pitch (DoubleRow group stride must be %16)
WSCALE = 64.0        # host-side prescale of conv weights/biases
EXP_SCALE = 1.0 / (16.0 * WSCALE * WSCALE)  # 1/sqrt(D) on 64x-unit scores

NP8 = ml_dtypes.float8_e4m3
NP85 = ml_dtypes.float8_e5m2

_ACT_DST = None


def _ensure_patched_tables():
    """Build an act-table dir where `exp` computes expm1, and point the
    NEFF compile at it via BASS_ACT_ROOT_JSON_PATH.

    The pwp bucket tables store per-section Taylor coefficients
    [d0,d1,d2,d3,x0] (f32 each, 32B stride). Subtracting 1.0 from every
    d0 of exp's 781 entries turns the piecewise cubic for exp into one
    for expm1 exactly (derivatives unchanged); the small-signal entry
    becomes x + x^2/2 + x^3/6 (full relative precision near 0), the
    positive saturation stays +inf, the negative one becomes -1.
    """
    global _ACT_DST
    if _ACT_DST is not None:
        return
    from neuronxcc.driver.Job import Job
    from neuronxcc.driver.jobs.support.FindActInfo import findActInfoFile

    src_json = findActInfoFile(Job.getPackageDir(), "gen3")
    src = os.path.dirname(src_json)
    dst = "/tmp/act_expm1_kernel"
    if not os.path.exists(os.path.join(dst, "act_info.json")):
        tmp = dst + ".tmp"
        if os.path.exists(tmp):
            shutil.rmtree(tmp)
        shutil.copytree(src, tmp)
        bp = os.path.join(tmp, "exp_and_others_bkt.bin")
        raw = bytearray(open(bp, "rb").read())
        for i in range(781):
            (d0,) = struct.unpack_from("<f", raw, i * 32)
            struct.pack_into("<f", raw, i * 32, np.float32(d0) - np.float32(1.0))
        open(bp, "wb").write(bytes(raw))
        jp = os.path.join(tmp, "exp_and_others.json")
        sj = json.load(open(jp))
        for meta in sj["profile_meta_data"]:
            if meta["func_name"].startswith("exp"):
                meta["fzero_result"] = 0           # expm1(0) = 0
                meta["fninf_result"] = 0xBF800000  # expm1(-inf) = -1
        json.dump(sj, open(jp, "w"))
        if os.path.exists(dst):
            shutil.rmtree(dst)
        os.replace(tmp, dst)
    os.environ["BASS_ACT_ROOT_JSON_PATH"] = os.path.join(dst, "act_info.json")
    _ACT_DST = dst


def _split_drain_waits(nc):
    """Walrus in this toolchain only accepts one sem-wait per CTRL (Drain)
    instruction; Tile's kernel-tail drain carries one wait per active proc.
    Split any multi-wait Drain into a chain of single-wait drains."""
    import concourse.mybir as mybir

    def walk(blocks):
        for b in blocks:
            insts = b.instructions
            i = 0
            while i < len(insts):
                inst = insts[i]
                si = getattr(inst, "sync_info", None)
                w = list(si.on_wait) if si is not None and si.on_wait else []
                if len(w) > 1:
                    pre = [
                        mybir.InstNoOp(
                            name=f"{inst.name}-ws{j}",
                            engine=inst.engine,
                            ins=[],
                            outs=[],
                            sync_info=mybir.SyncInfo(on_wait=[wj], on_update=[]),
                        )
                        for j, wj in enumerate(w[:-1])
                    ]
                    si.on_wait = w[-1:]
                    for k, nd in enumerate(pre):
                        insts.insert(i + k, nd)
                    i += len(pre)
                i += 1
            walk(getattr(b, "blocks", []) or [])

    for f in nc.m.functions:
        walk(f.blocks)


def _build_bass():
    import concourse.bass as bass
    import concourse.mybir as mybir
    import concourse.tile as tile

    f32 = mybir.dt.float32
    fp8 = mybir.dt.float8e4
    fp8e5 = mybir.dt.float8e5
    ADD = mybir.AluOpType.add
    SUB = mybir.AluOpType.subtract
    MULT = mybir.AluOpType.mult
    DR = mybir.MatmulPerfMode.DoubleRow
    EXP = mybir.ActivationFunctionType.Exp  # table patched to expm1

    nc = bass.Bass(trn_type="TRN2")

    # ---- DRAM I/O (per-core shard shapes); dim1 = {hi, lo} fp8 pair ----
    xq = nc.dram_tensor("xq", [128, 2, DCH, SQ + 2], fp8, kind="ExternalInput")
    xk = nc.dram_tensor("xk", [128, 2, DCH, SK + 2], fp8, kind="ExternalInput")
    xv = nc.dram_tensor("xv", [128, 2, DCH, SK + 2], fp8, kind="ExternalInput")
    # [p, hl, ci_chunk a, tap t, co_chunk o, co_in_chunk m]
    wq = nc.dram_tensor("wq", [128, 2, DCH, 3, DCH, 128], fp8, kind="ExternalInput")
    wk = nc.dram_tensor("wk", [128, 2, DCH, 3, DCH, 128], fp8, kind="ExternalInput")
    # [p, hl, ci_chunk a, tap t, co]
    wv = nc.dram_tensor("wv", [128, 2, DCH, 3, D], fp8, kind="ExternalInput")
    bq = nc.dram_tensor("bq", [128, DCH], f32, kind="ExternalInput")
    bk = nc.dram_tensor("bk", [128, DCH], f32, kind="ExternalInput")
    bvb = nc.dram_tensor("bvb", [128, D], f32, kind="ExternalInput")
    scl = nc.dram_tensor("scl", [128, 1], f32, kind="ExternalInput")
    out = nc.dram_tensor("out", [SQ // 128, 128, D], f32, kind="ExternalOutput")

    TERMS = ((0, 0), (0, 1), (1, 0))  # (x hl index, w hl index): hh, hl, lh

    with tile.TileContext(nc) as tc:
        with (
            tc.tile_pool(name="persist", bufs=1) as persist,
            tc.tile_pool(name="xin", bufs=3) as xin,
            tc.tile_pool(name="epool", bufs=3) as epool,
            tc.tile_pool(name="vstg", bufs=3) as vstg,
            tc.tile_pool(name="outp", bufs=3) as outp,
            tc.tile_pool(name="tiny", bufs=4) as tiny,
            tc.tile_pool(name="ps_conv", bufs=2, space="PSUM") as ps_conv,
            tc.tile_pool(name="ps_sc", bufs=3, space="PSUM") as ps_sc,
        ):
            # ---- persistent SBUF ----
            wq_s = persist.tile([128, 2, DCH, 3, DCH, 128], fp8, tag="wq_s")
            wk_s = persist.tile([128, 2, DCH, 3, DCH, 128], fp8, tag="wk_s")
            wv_s = persist.tile([128, 2, DCH, 3, D], fp8, tag="wv_s")
            bq_s = persist.tile([128, DCH], f32, tag="bq_s")
            bk_s = persist.tile([128, DCH], f32, tag="bk_s")
            bvb_s = persist.tile([128, D], f32, tag="bvb_s")
            scl_s = persist.tile([128, 1], f32, tag="scl_s")
            qT8 = persist.tile([128, DCH, SQ], fp8, tag="qT8")
            kT8 = persist.tile([128, DCH, SK], fp8, tag="kT8")
            vh_s = persist.tile([128, NKT, VROW], fp8, tag="vh_s")
            vl_s = persist.tile([128, NKT, VROW], fp8e5, tag="vl_s")
            ones8 = persist.tile([128, 2, 128], fp8, tag="ones8")
            sv_sb = persist.tile([128, D + 1], f32, tag="sv_sb")

            # denominator column of v_aug: vh gets 1.0, vl gets 0.0
            nc.vector.memset(vh_s[:, :, D : D + 1], 1.0)
            nc.vector.memset(vl_s[:, :, D : D + 1], 0.0)
            nc.vector.memset(ones8[:], 1.0)

            def conv_T_tile(x_dram, w_s, b_s, out_s, j, tag):
                """T-layout conv col tile j: out_s[:, o, jCT:(j+1)CT] fp8."""
                xt = xin.tile([128, 2, DCH, CT + 16], fp8, tag=tag, name=tag)
                nc.sync.dma_start(
                    xt[:, :, :, 0 : CT + 2],
                    x_dram[:, :, :, j * CT : j * CT + CT + 2],
                )
                for o in range(DCH):
                    ps = ps_conv.tile([128, CT], f32, tag="ps_T", name="ps_T")
                    n = 0
                    for xi, wi in TERMS:
                        for t in range(3):
                            nc.tensor.matmul(
                                ps[:],
                                w_s[:, wi, :, t, o, :],
                                xt[:, xi, :, t : t + CT],
                                start=(n == 0),
                                stop=(n == 8),
                                perf_mode=DR,
                            )
                            n += 1
                    nc.vector.tensor_scalar(
                        out=out_s[:, o, j * CT : (j + 1) * CT],
                        in0=ps[:],
                        scalar1=b_s[:, o : o + 1],
                        scalar2=None,
                        op0=ADD,
                    )

            def conv_v_row(xt, j, rr):
                """v conv rowtile -> f32 bias-add (DVE) -> hi/lo fp8 (Pool)."""
                kt = j * (CT // 128) + rr
                ps = ps_conv.tile([128, CT], f32, tag="ps_T", name="ps_T")
                n = 0
                for xi, wi in TERMS:
                    for t in range(3):
                        nc.tensor.matmul(
                            ps[:, 0:D],
                            xt[:, xi, :, t + rr * 128 : t + rr * 128 + 128],
                            wv_s[:, wi, :, t, :],
                            start=(n == 0),
                            stop=(n == 8),
                            perf_mode=DR,
                        )
                        n += 1
                vb = vstg.tile([128, D], f32, tag="vb", name="vb")
                nc.vector.tensor_tensor(
                    out=vb[:], in0=ps[:, 0:D], in1=bvb_s[:], op=ADD
                )
                nc.gpsimd.tensor_copy(out=vh_s[:, kt, 0:D], in_=vb[:])
                nc.gpsimd.tensor_tensor(
                    out=vl_s[:, kt, 0:D], in0=vb[:], in1=vh_s[:, kt, 0:D], op=SUB
                )

            # ---- prefix: weights ride the idle Act HWDGE queue; x tiles
            # own SP from t=0 ----
            nc.scalar.dma_start(wq_s[:], wq[:])
            nc.scalar.dma_start(wk_s[:], wk[:])
            nc.scalar.dma_start(wv_s[:], wv[:])
            nc.scalar.dma_start(bvb_s[:], bvb[:])
            nc.scalar.dma_start(scl_s[:], scl[:])
            nc.sync.dma_start(bq_s[:], bq[:])
            conv_T_tile(xq, wq_s, bq_s, qT8, 0, "xt_q")
            nc.sync.dma_start(bk_s[:], bk[:])
            conv_T_tile(xk, wk_s, bk_s, kT8, 0, "xt_k")
            conv_T_tile(xk, wk_s, bk_s, kT8, 1, "xt_k")

            # ---- filler units interleaved into block 0's mm1 stream ----
            kf = [
                (lambda j=j: conv_T_tile(xk, wk_s, bk_s, kT8, j, "xt_k"))
                for j in range(2, SK // CT)
            ]
            kf.reverse()

            vxts = [None] * (SK // CT)

            def vdma(j):
                xt = xin.tile(
                    [128, 2, DCH, CT + 16], fp8, tag="xt_v", name="xt_v",
                    bufs=SK // CT,
                )
                nc.sync.dma_start(
                    xt[:, :, :, 0 : CT + 2],
                    xv[:, :, :, j * CT : j * CT + CT + 2],
                )
                vxts[j] = xt

            def vrow(r):
                conv_v_row(vxts[r // 4], r // 4, r % 4)

            e_bufs = []

            def mm1_pair(b, eb, i):
                """scores^T + expm1 for kt pair (2i, 2i+1) of query block b."""
                ps = ps_sc.tile([128, 1024], f32, tag="ps_sc", name="ps_sc")
                for h in range(2):
                    kt = 2 * i + h
                    nc.tensor.matmul(
                        ps[:, h * 512 : h * 512 + 512],
                        kT8[:, :, kt * 128 : (kt + 1) * 128],
                        qT8[:, :, b * QB : (b + 1) * QB],
                        start=True,
                        stop=True,
                        perf_mode=DR,
                    )
                nc.scalar.activation(
                    out=eb[:, 2 * i : 2 * i + 2, :],
                    in_=ps[:],
                    func=EXP,
                    scale=EXP_SCALE,
                )

            # mm2: 4 chunk-parts per qs tile, each 8 DoubleRow kt-pair matmuls
            # parts 0,1 = vh halves; parts 2,3 = vl halves (one psum group)
            states = {}

            def mm2_chunk(b, i):
                eb = e_bufs[b]
                state = states.setdefault(b, {})
                qs, part = divmod(i, 4)
                if part == 0:
                    state["po"] = ps_conv.tile(
                        [128, 512], f32, tag="ps_T", name="ps_T"
                    )
                po = state["po"]
                vt = vh_s if part < 2 else vl_s
                kp0 = 8 * (part % 2)
                for kp in range(kp0, kp0 + 8):
                    nc.tensor.matmul(
                        po[:, 0:DV],
                        eb[:, 2 * kp : 2 * kp + 2, qs * 128 : (qs + 1) * 128],
                        vt[:, 2 * kp : 2 * kp + 2, 0:DV],
                        start=(part == 0 and kp == kp0),
                        stop=(part == 3 and kp == kp0 + 7),
                        perf_mode=DR,
                    )
                if part == 3:
                    tot = outp.tile([128, D + 1], f32, tag="tot", name="tot")
                    nc.vector.tensor_tensor(
                        out=tot[:], in0=po[:, 0 : D + 1], in1=sv_sb[:], op=ADD
                    )
                    rec = tiny.tile([128, 1], f32, tag="rec", name="rec")
                    nc.vector.reciprocal(rec[:], tot[:, D : D + 1])
                    nc.vector.tensor_tensor(
                        out=rec[:], in0=rec[:], in1=scl_s[:], op=MULT
                    )
                    ot = outp.tile([128, D], f32, tag="ot", name="ot")
                    nc.vector.tensor_scalar_mul(ot[:], tot[:, 0:D], rec[:])
                    nc.sync.dma_start(out[b * (QB // 128) + qs, :, :], ot[:])

            def emit_sv():
                sv_ps = ps_conv.tile([128, CT], f32, tag="ps_T", name="ps_T")
                n = 0
                for vt in (vh_s, vl_s):
                    for kp in range(NKT // 2):
                        nc.tensor.matmul(
                            sv_ps[:, 0:DV],
                            ones8[:],
                            vt[:, 2 * kp : 2 * kp + 2, 0:DV],
                            start=(n == 0),
                            stop=(n == NKT - 1),
                            perf_mode=DR,
                        )
                        n += 1
                nc.vector.tensor_copy(out=sv_sb[:], in_=sv_ps[:, 0 : D + 1])

            # ---- attention pipeline ----
            # window 0: mm1(0) paced against k-conv tiles; v DMAs prefetch;
            # q-conv tile b+1 lands at the end of window b (before the
            # pairs of window b+1 that read it)
            eb0 = epool.tile([128, NKT, QB], fp8, tag="e_s", name="e_s")
            e_bufs.append(eb0)
            for i in range(NKT // 2):
                mm1_pair(0, eb0, i)
                if i % 2 == 1 and kf:
                    kf.pop()()
                if i < SK // CT:
                    vdma(i)
            conv_T_tile(xq, wq_s, bq_s, qT8, 1, "xt_q")

            # per-pair unit schedules for windows 1..3 (PE work under the
            # ~17.5us exp budget per window); mm2(b) runs 2-3 windows after
            # its exp so v-conv can fill window 1
            def U_vrow(r):
                return lambda: vrow(r)

            def U_chunk(b, j):
                return lambda: mm2_chunk(b, j)

            UNITS = {1: [[] for _ in range(16)], 2: [[] for _ in range(16)],
                     3: [[] for _ in range(16)]}
            for r in range(24):          # w1: v rows 0..23, 1-2 per pair
                UNITS[1][(r * 16) // 24].append(U_vrow(r))
            for r in range(24, 32):      # w2: v rows 24..31 on pairs 0..3
                UNITS[2][(r - 24) // 2].append(U_vrow(r))
            UNITS[2][4].append(emit_sv)
            for n, (bb, j) in enumerate([(0, j) for j in range(14)]):
                UNITS[2][5 + (n * 11) // 14].append(U_chunk(bb, j))
            W3 = ([(0, 14), (0, 15)] + [(1, j) for j in range(16)]
                  + [(2, j) for j in range(14)])
            for n, (bb, j) in enumerate(W3):
                UNITS[3][(n * 16) // len(W3)].append(U_chunk(bb, j))

            for b in range(1, NBLK):
                eb = epool.tile([128, NKT, QB], fp8, tag="e_s", name="e_s")
                e_bufs.append(eb)
                for i in range(NKT // 2):
                    mm1_pair(b, eb, i)
                    for u in UNITS[b][i]:
                        u()
                if b < NBLK - 1:
                    conv_T_tile(xq, wq_s, bq_s, qT8, b + 1, "xt_q")
            for bb, j in [(2, 14), (2, 15)] + [(3, j) for j in range(16)]:
                mm2_chunk(bb, j)

    _split_drain_waits(nc)
    return nc


_NC_CACHE = None


def _get_nc():
    global _NC_CACHE
    if _NC_CACHE is None:
        _NC_CACHE = _build_bass()
    return _NC_CACHE


def _split8(a):
    h = a.astype(NP8)
    l = (a - h.astype(np.float32)).astype(NP8)
    return h, l


def _prep_shared(q_w, q_b, k_w, k_b, v_w, v_b, scale):
    def w_T(w):  # [co, ci, 3] -> [p, hl, a, t, o, m] fp8 of 64*w
        arr = np.ascontiguousarray((w * WSCALE).transpose(1, 2, 0)).astype(np.float32)
        arr = arr.reshape(DCH, 128, 3, DCH, 128).transpose(1, 0, 2, 3, 4)
        h, l = _split8(arr)
        return np.ascontiguousarray(np.stack([h, l], axis=1))

    def w_v(w):  # [co, ci, 3] -> [p, hl, a, t, co] fp8 of 64*w
        arr = np.ascontiguousarray((w * WSCALE).transpose(1, 2, 0)).astype(np.float32)
        arr = arr.reshape(DCH, 128, 3, D).transpose(1, 0, 2, 3)
        h, l = _split8(arr)
        return np.ascontiguousarray(np.stack([h, l], axis=1))

    def b_T(b):  # [co] -> [p, o] of 64*b
        return np.ascontiguousarray(
            (np.asarray(b, np.float64) * WSCALE).astype(np.float32).reshape(DCH, 128).T
        )

    return {
        "wq": w_T(q_w),
        "wk": w_T(k_w),
        "wv": w_v(v_w),
        "bq": b_T(q_b),
        "bk": b_T(k_b),
        "bvb": np.ascontiguousarray(
            np.tile((np.asarray(v_b, np.float32) * np.float32(WSCALE)), (128, 1))
        ),
        "scl": np.full((128, 1), float(scale) / WSCALE, np.float32),
    }


def _xT_pad8(x_b):
    """[S, C] -> transposed+halo-padded hi/lo fp8 [128, 2, DCH, S+2]."""
    xt = np.zeros((DIN, x_b.shape[0] + 2), np.float32)
    xt[:, 1:-1] = x_b.T
    h, l = _split8(xt)

    def lay(a):
        return a.reshape(DCH, 128, -1).transpose(1, 0, 2)

    return np.ascontiguousarray(np.stack([lay(h), lay(l)], axis=1))


def kernel(query, key, value, q_w, q_b, k_w, k_b, v_w, v_b, scale):
    _ensure_patched_tables()
    from concourse.bass_utils import run_bass_kernel_spmd

    query = np.asarray(query, np.float32)
    key = np.asarray(key, np.float32)
    value = np.asarray(value, np.float32)

    shared = _prep_shared(
        np.asarray(q_w), np.asarray(q_b), np.asarray(k_w), np.asarray(k_b),
        np.asarray(v_w), np.asarray(v_b), np.asarray(scale),
    )

    in_maps = []
    for c in range(NCORES):
        b, h = c // 2, c % 2
        xq_full = _xT_pad8(query[b])  # [128, 2, DCH, S+2]
        xq_c = np.ascontiguousarray(xq_full[:, :, :, h * SQ : h * SQ + SQ + 2])
        m = dict(shared)
        m["xq"] = xq_c
        m["xk"] = _xT_pad8(key[b])
        m["xv"] = _xT_pad8(value[b])
        in_maps.append(m)

    nc = _get_nc()
    res = run_bass_kernel_spmd(nc, in_maps, core_ids=list(range(NCORES)))

    out_full = np.empty((B, S, D), np.float32)
    for c in range(NCORES):
        b, h = c // 2, c % 2
        out_full[b, h * SQ : (h + 1) * SQ, :] = res.results[c]["out"].reshape(SQ, D)
    return out_full


# revision 21
# speedup vs baseline: 1.0058x; 1.0058x over previous
"""Conv1d-QKV + full attention kernel for TRN2, 8 NeuronCores.

Problem (hardcoded shapes): B=4, S=4096, DIN=DQ=DK=256.
  q = conv1d(query, q_w, q_b); k = conv1d(key, ...); v = conv1d(value, ...)
  out = scale * softmax(q @ k^T / sqrt(256)) @ v

Sharding: 8 cores = (batch b = core//2) x (query half h = core%2).
Each core computes k/v conv over the full 4096 rows of its batch
(replicated across the 2 cores of a batch) and attention for its 2048
query rows.

All matmul speed comes from fp8e4m3/e5m2 DoubleRow at 0.5 cycles/row
(contracts 2x128 per instruction, 4x the f32r rate):
  - convs: hi/lo fp8 split of inputs and (64x prescaled) weights on the
    host; 3-term product xh*wh + xh*wl + xl*wh per tap (~0.15% error).
  - scores mm: q,k cast to fp8e4m3 (in 64x units) by the conv bias-add
    on DVE; DoubleRow over ci chunks. Dominant error term (~1e-2
    scale-relative; gate is 2e-2).
  - softmax weights are CENTERED before quantization: the activation
    computes expm1(s/16) via a patched act table (exp with 1.0
    subtracted from every d0 Taylor coefficient), so fp8's ~2.7%
    relative noise applies to e-1 (std ~0.37) instead of e (~1.1) --
    3x less noise. out = scale*(Sv + e_dev^T v)/(4096 + sum(e_dev)),
    where Sv = column sums of v from one extra DoubleRow chain.
  - out mm: v split into vh (e4m3) + vl (e5m2 residual); both fp8
    DoubleRow passes against e_dev. The Pool engine does the split.
Scale folds are exact powers of two: no extra ops, no extra error.
"""

import os
import json
import shutil
import struct

import numpy as np
import ml_dtypes

B, S, DIN, D = 4, 4096, 256, 256
NCORES = 8
SQ = S // 2          # query rows per core
SK = S               # key rows per core
DCH = 2              # 128-partition chunks of D / DIN
CT = 512             # conv column tile
QB = 512             # query block for attention
NBLK = SQ // QB      # 4
NKT = SK // 128      # 32 k tiles
DV = D + 4           # v_aug cols: D of v + [1,0,0,0] denominator col
VROW = 272           # v row pitch (DoubleRow group stride must be %16)
WSCALE = 64.0        # host-side prescale of conv weights/biases
EXP_SCALE = 1.0 / (16.0 * WSCALE * WSCALE)  # 1/sqrt(D) on 64x-unit scores

NP8 = ml_dtypes.float8_e4m3
NP85 = ml_dtypes.float8_e5m2

_ACT_DST = None


def _ensure_patched_tables():
    """Build an act-table dir where `exp` computes expm1, and point the
    NEFF compile at it via BASS_ACT_ROOT_JSON_PATH.

    The pwp bucket tables store per-section Taylor coefficients
    [d0,d1,d2,d3,x0] (f32 each, 32B stride). Subtracting 1.0 from every
    d0 of exp's 781 entries turns the piecewise cubic for exp into one
    for expm1 exactly (derivatives unchanged); the small-signal entry
    becomes x + x^2/2 + x^3/6 (full relative precision near 0), the
    positive saturation stays +inf, the negative one becomes -1.
    """
    global _ACT_DST
    if _ACT_DST is not None:
        return
    from neuronxcc.driver.Job import Job
    from neuronxcc.driver.jobs.support.FindActInfo import findActInfoFile

    src_json = findActInfoFile(Job.getPackageDir(), "gen3")
    src = os.path.dirname(src_json)
    dst = "/tmp/act_expm1_kernel"
    if not os.path.exists(os.path.join(dst, "act_info.json")):
        tmp = f"{dst}.tmp.{os.getpid()}"
        if os.path.exists(tmp):
            shutil.rmtree(tmp)
        shutil.copytree(src, tmp)
        bp = os.path.join(tmp, "exp_and_others_bkt.bin")
        raw = bytearray(open(bp, "rb").read())
        for i in range(781):
            (d0,) = struct.unpack_from("<f", raw, i * 32)
            struct.pack_into("<f", raw, i * 32, np.float32(d0) - np.float32(1.0))
        open(bp, "wb").write(bytes(raw))
        jp = os.path.join(tmp, "exp_and_others.json")
        sj = json.load(open(jp))
        for meta in sj["profile_meta_data"]:
            if meta["func_name"].startswith("exp"):
                meta["fzero_result"] = 0           # expm1(0) = 0
                meta["fninf_result"] = 0xBF800000  # expm1(-inf) = -1
        json.dump(sj, open(jp, "w"))
        if os.path.exists(dst):
            shutil.rmtree(dst)
        os.replace(tmp, dst)
    os.environ["BASS_ACT_ROOT_JSON_PATH"] = os.path.join(dst, "act_info.json")
    _ACT_DST = dst


def _split_drain_waits(nc):
    """Walrus in this toolchain only accepts one sem-wait per CTRL (Drain)
    instruction; Tile's kernel-tail drain carries one wait per active proc.
    Split any multi-wait Drain into a chain of single-wait drains."""
    import concourse.mybir as mybir

    def walk(blocks):
        for b in blocks:
            insts = b.instructions
            i = 0
            while i < len(insts):
                inst = insts[i]
                si = getattr(inst, "sync_info", None)
                w = list(si.on_wait) if si is not None and si.on_wait else []
                if len(w) > 1:
                    pre = [
                        mybir.InstNoOp(
                            name=f"{inst.name}-ws{j}",
                            engine=inst.engine,
                            ins=[],
                            outs=[],
                            sync_info=mybir.SyncInfo(on_wait=[wj], on_update=[]),
                        )
                        for j, wj in enumerate(w[:-1])
                    ]
                    si.on_wait = w[-1:]
                    for k, nd in enumerate(pre):
                        insts.insert(i + k, nd)
                    i += len(pre)
                i += 1
            walk(getattr(b, "blocks", []) or [])

    for f in nc.m.functions:
        walk(f.blocks)


def _build_bass():
    import concourse.bass as bass
    import concourse.mybir as mybir
    import concourse.tile as tile

    f32 = mybir.dt.float32
    fp8 = mybir.dt.float8e4
    fp8e5 = mybir.dt.float8e5
    ADD = mybir.AluOpType.add
    SUB = mybir.AluOpType.subtract
    MULT = mybir.AluOpType.mult
    DR = mybir.MatmulPerfMode.DoubleRow
    EXP = mybir.ActivationFunctionType.Exp  # table patched to expm1

    nc = bass.Bass(trn_type="TRN2")

    # ---- DRAM I/O (per-core shard shapes); dim1 = {hi, lo} fp8 pair ----
    xq = nc.dram_tensor("xq", [128, 2, DCH, SQ + 2], fp8, kind="ExternalInput")
    xk = nc.dram_tensor("xk", [128, 2, DCH, SK + 2], fp8, kind="ExternalInput")
    xv = nc.dram_tensor("xv", [128, 2, DCH, SK + 2], fp8, kind="ExternalInput")
    # [p, hl, ci_chunk a, tap t, co_chunk o, co_in_chunk m]
    wq = nc.dram_tensor("wq", [128, 2, DCH, 3, DCH, 128], fp8, kind="ExternalInput")
    wk = nc.dram_tensor("wk", [128, 2, DCH, 3, DCH, 128], fp8, kind="ExternalInput")
    # [p, hl, ci_chunk a, tap t, co]
    wv = nc.dram_tensor("wv", [128, 2, DCH, 3, D], fp8, kind="ExternalInput")
    bq = nc.dram_tensor("bq", [128, DCH], f32, kind="ExternalInput")
    bk = nc.dram_tensor("bk", [128, DCH], f32, kind="ExternalInput")
    bvb = nc.dram_tensor("bvb", [128, D], f32, kind="ExternalInput")
    scl = nc.dram_tensor("scl", [128, 1], f32, kind="ExternalInput")
    out = nc.dram_tensor("out", [SQ // 128, 128, D], f32, kind="ExternalOutput")

    TERMS = ((0, 0), (0, 1), (1, 0))  # (x hl index, w hl index): hh, hl, lh

    with tile.TileContext(nc) as tc:
        with (
            tc.tile_pool(name="persist", bufs=1) as persist,
            tc.tile_pool(name="xin", bufs=3) as xin,
            tc.tile_pool(name="epool", bufs=4) as epool,
            tc.tile_pool(name="vstg", bufs=3) as vstg,
            tc.tile_pool(name="outp", bufs=3) as outp,
            tc.tile_pool(name="tiny", bufs=4) as tiny,
            tc.tile_pool(name="ps_conv", bufs=2, space="PSUM") as ps_conv,
            tc.tile_pool(name="ps_sc", bufs=3, space="PSUM") as ps_sc,
        ):
            # ---- persistent SBUF ----
            wq_s = persist.tile([128, 2, DCH, 3, DCH, 128], fp8, tag="wq_s")
            wk_s = persist.tile([128, 2, DCH, 3, DCH, 128], fp8, tag="wk_s")
            wv_s = persist.tile([128, 2, DCH, 3, D], fp8, tag="wv_s")
            bq_s = persist.tile([128, DCH], f32, tag="bq_s")
            bk_s = persist.tile([128, DCH], f32, tag="bk_s")
            bvb_s = persist.tile([128, D], f32, tag="bvb_s")
            scl_s = persist.tile([128, 1], f32, tag="scl_s")
            qT8 = persist.tile([128, DCH, SQ], fp8, tag="qT8")
            kT8 = persist.tile([128, DCH, SK], fp8, tag="kT8")
            vh_s = persist.tile([128, NKT, VROW], fp8, tag="vh_s")
            vl_s = persist.tile([128, NKT, VROW], fp8e5, tag="vl_s")
            ones8 = persist.tile([128, 2, 128], fp8, tag="ones8")
            sv_sb = persist.tile([128, D + 1], f32, tag="sv_sb")

            # denominator column of v_aug: vh gets 1.0, vl gets 0.0
            nc.vector.memset(vh_s[:, :, D : D + 1], 1.0)
            nc.vector.memset(vl_s[:, :, D : D + 1], 0.0)
            nc.vector.memset(ones8[:], 1.0)

            def conv_T_tile(x_dram, w_s, b_s, out_s, j, tag):
                """T-layout conv col tile j: out_s[:, o, jCT:(j+1)CT] fp8."""
                xt = xin.tile([128, 2, DCH, CT + 16], fp8, tag=tag, name=tag)
                nc.sync.dma_start(
                    xt[:, :, :, 0 : CT + 2],
                    x_dram[:, :, :, j * CT : j * CT + CT + 2],
                )
                for o in range(DCH):
                    ps = ps_conv.tile([128, CT], f32, tag="ps_T", name="ps_T")
                    n = 0
                    for xi, wi in TERMS:
                        for t in range(3):
                            nc.tensor.matmul(
                                ps[:],
                                w_s[:, wi, :, t, o, :],
                                xt[:, xi, :, t : t + CT],
                                start=(n == 0),
                                stop=(n == 8),
                                perf_mode=DR,
                            )
                            n += 1
                    nc.vector.tensor_scalar(
                        out=out_s[:, o, j * CT : (j + 1) * CT],
                        in0=ps[:],
                        scalar1=b_s[:, o : o + 1],
                        scalar2=None,
                        op0=ADD,
                    )

            def conv_v_row(xt, j, rr):
                """v conv rowtile -> f32 bias-add (DVE) -> hi/lo fp8 (Pool)."""
                kt = j * (CT // 128) + rr
                ps = ps_conv.tile([128, CT], f32, tag="ps_T", name="ps_T")
                n = 0
                for xi, wi in TERMS:
                    for t in range(3):
                        nc.tensor.matmul(
                            ps[:, 0:D],
                            xt[:, xi, :, t + rr * 128 : t + rr * 128 + 128],
                            wv_s[:, wi, :, t, :],
                            start=(n == 0),
                            stop=(n == 8),
                            perf_mode=DR,
                        )
                        n += 1
                vb = vstg.tile([128, D], f32, tag="vb", name="vb")
                nc.vector.tensor_tensor(
                    out=vb[:], in0=ps[:, 0:D], in1=bvb_s[:], op=ADD
                )
                nc.gpsimd.tensor_copy(out=vh_s[:, kt, 0:D], in_=vb[:])
                nc.gpsimd.tensor_tensor(
                    out=vl_s[:, kt, 0:D], in0=vb[:], in1=vh_s[:, kt, 0:D], op=SUB
                )

            # ---- prefix: weights ride the idle Act HWDGE queue; x tiles
            # own SP from t=0 ----
            nc.scalar.dma_start(wq_s[:], wq[:])
            nc.scalar.dma_start(wk_s[:], wk[:])
            nc.scalar.dma_start(wv_s[:], wv[:])
            nc.scalar.dma_start(bvb_s[:], bvb[:])
            nc.scalar.dma_start(scl_s[:], scl[:])
            nc.sync.dma_start(bq_s[:], bq[:])
            conv_T_tile(xq, wq_s, bq_s, qT8, 0, "xt_q")
            nc.sync.dma_start(bk_s[:], bk[:])
            conv_T_tile(xk, wk_s, bk_s, kT8, 0, "xt_k")

            # ---- filler units interleaved into block 0's mm1 stream ----
            kf = [
                (lambda j=j: conv_T_tile(xk, wk_s, bk_s, kT8, j, "xt_k"))
                for j in range(1, SK // CT)
            ]
            kf.reverse()

            vxts = [None] * (SK // CT)

            def vdma(j):
                xt = xin.tile(
                    [128, 2, DCH, CT + 16], fp8, tag="xt_v", name="xt_v",
                    bufs=SK // CT,
                )
                nc.sync.dma_start(
                    xt[:, :, :, 0 : CT + 2],
                    xv[:, :, :, j * CT : j * CT + CT + 2],
                )
                vxts[j] = xt

            def vrow(r):
                conv_v_row(vxts[r // 4], r // 4, r % 4)

            e_bufs = []

            def mm1_pair(b, eb, i):
                """scores^T + expm1 for kt pair (2i, 2i+1) of query block b."""
                ps = ps_sc.tile([128, 1024], f32, tag="ps_sc", name="ps_sc")
                for h in range(2):
                    kt = 2 * i + h
                    nc.tensor.matmul(
                        ps[:, h * 512 : h * 512 + 512],
                        kT8[:, :, kt * 128 : (kt + 1) * 128],
                        qT8[:, :, b * QB : (b + 1) * QB],
                        start=True,
                        stop=True,
                        perf_mode=DR,
                    )
                nc.scalar.activation(
                    out=eb[:, 2 * i : 2 * i + 2, :],
                    in_=ps[:],
                    func=EXP,
                    scale=EXP_SCALE,
                )

            # mm2: 4 chunk-parts per qs tile, each 8 DoubleRow kt-pair matmuls
            # parts 0,1 = vh halves; parts 2,3 = vl halves (one psum group)
            states = {}

            def mm2_chunk(b, i):
                eb = e_bufs[b]
                state = states.setdefault(b, {})
                qs, part = divmod(i, 4)
                if part == 0:
                    state["po"] = ps_conv.tile(
                        [128, 512], f32, tag="ps_T", name="ps_T"
                    )
                po = state["po"]
                vt = vh_s if part < 2 else vl_s
                kp0 = 8 * (part % 2)
                for kp in range(kp0, kp0 + 8):
                    nc.tensor.matmul(
                        po[:, 0:DV],
                        eb[:, 2 * kp : 2 * kp + 2, qs * 128 : (qs + 1) * 128],
                        vt[:, 2 * kp : 2 * kp + 2, 0:DV],
                        start=(part == 0 and kp == kp0),
                        stop=(part == 3 and kp == kp0 + 7),
                        perf_mode=DR,
                    )
                if part == 3:
                    tot = outp.tile([128, D + 1], f32, tag="tot", name="tot")
                    nc.vector.tensor_tensor(
                        out=tot[:], in0=po[:, 0 : D + 1], in1=sv_sb[:], op=ADD
                    )
                    rec = tiny.tile([128, 1], f32, tag="rec", name="rec")
                    nc.vector.reciprocal(rec[:], tot[:, D : D + 1])
                    nc.vector.tensor_tensor(
                        out=rec[:], in0=rec[:], in1=scl_s[:], op=MULT
                    )
                    ot = outp.tile([128, D], f32, tag="ot", name="ot")
                    nc.vector.tensor_scalar_mul(ot[:], tot[:, 0:D], rec[:])
                    nc.sync.dma_start(out[b * (QB // 128) + qs, :, :], ot[:])

            def emit_sv():
                sv_ps = ps_conv.tile([128, CT], f32, tag="ps_T", name="ps_T")
                n = 0
                for vt in (vh_s, vl_s):
                    for kp in range(NKT // 2):
                        nc.tensor.matmul(
                            sv_ps[:, 0:DV],
                            ones8[:],
                            vt[:, 2 * kp : 2 * kp + 2, 0:DV],
                            start=(n == 0),
                            stop=(n == NKT - 1),
                            perf_mode=DR,
                        )
                        n += 1
                nc.vector.tensor_copy(out=sv_sb[:], in_=sv_ps[:, 0 : D + 1])

            # ---- attention pipeline ----
            # window 0: mm1(0) paced against k-conv tiles; v DMAs prefetch;
            # q-conv tile b+1 lands at the end of window b (before the
            # pairs of window b+1 that read it)
            eb0 = epool.tile([128, NKT, QB], fp8, tag="e_s", name="e_s")
            e_bufs.append(eb0)
            for i in range(NKT // 2):
                mm1_pair(0, eb0, i)
                if kf:
                    kf.pop()()
                if i >= 8:
                    vdma(i - 8)
            conv_T_tile(xq, wq_s, bq_s, qT8, 1, "xt_q")

            # per-pair unit schedules for windows 1..3 (PE work under the
            # ~17.5us exp budget per window); mm2(b) runs 2-3 windows after
            # its exp so v-conv can fill window 1
            def U_vrow(r):
                return lambda: vrow(r)

            def U_chunk(b, j):
                return lambda: mm2_chunk(b, j)

            UNITS = {1: [[] for _ in range(16)], 2: [[] for _ in range(16)],
                     3: [[] for _ in range(16)]}
            for r in range(24):          # w1: v rows 0..23, 1-2 per pair
                UNITS[1][(r * 16) // 24].append(U_vrow(r))
            for r in range(24, 32):      # w2: v rows 24..31 on pairs 0..3
                UNITS[2][(r - 24) // 2].append(U_vrow(r))
            UNITS[2][4].append(emit_sv)
            for n, (bb, j) in enumerate([(0, j) for j in range(16)]):
                UNITS[2][5 + (n * 11) // 16].append(U_chunk(bb, j))
            W3 = [(1, j) for j in range(16)] + [(2, j) for j in range(16)]
            for n, (bb, j) in enumerate(W3):
                UNITS[3][(n * 16) // len(W3)].append(U_chunk(bb, j))

            for b in range(1, NBLK):
                eb = epool.tile([128, NKT, QB], fp8, tag="e_s", name="e_s")
                e_bufs.append(eb)
                for i in range(NKT // 2):
                    mm1_pair(b, eb, i)
                    for u in UNITS[b][i]:
                        u()
                if b < NBLK - 1:
                    conv_T_tile(xq, wq_s, bq_s, qT8, b + 1, "xt_q")
            for bb, j in [(3, j) for j in range(16)]:
                mm2_chunk(bb, j)

    _split_drain_waits(nc)
    return nc


_NC_CACHE = None


def _get_nc():
    global _NC_CACHE
    if _NC_CACHE is None:
        _NC_CACHE = _build_bass()
    return _NC_CACHE


def _split8(a):
    h = a.astype(NP8)
    l = (a - h.astype(np.float32)).astype(NP8)
    return h, l


def _prep_shared(q_w, q_b, k_w, k_b, v_w, v_b, scale):
    def w_T(w):  # [co, ci, 3] -> [p, hl, a, t, o, m] fp8 of 64*w
        arr = np.ascontiguousarray((w * WSCALE).transpose(1, 2, 0)).astype(np.float32)
        arr = arr.reshape(DCH, 128, 3, DCH, 128).transpose(1, 0, 2, 3, 4)
        h, l = _split8(arr)
        return np.ascontiguousarray(np.stack([h, l], axis=1))

    def w_v(w):  # [co, ci, 3] -> [p, hl, a, t, co] fp8 of 64*w
        arr = np.ascontiguousarray((w * WSCALE).transpose(1, 2, 0)).astype(np.float32)
        arr = arr.reshape(DCH, 128, 3, D).transpose(1, 0, 2, 3)
        h, l = _split8(arr)
        return np.ascontiguousarray(np.stack([h, l], axis=1))

    def b_T(b):  # [co] -> [p, o] of 64*b
        return np.ascontiguousarray(
            (np.asarray(b, np.float64) * WSCALE).astype(np.float32).reshape(DCH, 128).T
        )

    return {
        "wq": w_T(q_w),
        "wk": w_T(k_w),
        "wv": w_v(v_w),
        "bq": b_T(q_b),
        "bk": b_T(k_b),
        "bvb": np.ascontiguousarray(
            np.tile((np.asarray(v_b, np.float32) * np.float32(WSCALE)), (128, 1))
        ),
        "scl": np.full((128, 1), float(scale) / WSCALE, np.float32),
    }


def _xT_pad8(x_b):
    """[S, C] -> transposed+halo-padded hi/lo fp8 [128, 2, DCH, S+2]."""
    xt = np.zeros((DIN, x_b.shape[0] + 2), np.float32)
    xt[:, 1:-1] = x_b.T
    h, l = _split8(xt)

    def lay(a):
        return a.reshape(DCH, 128, -1).transpose(1, 0, 2)

    return np.ascontiguousarray(np.stack([lay(h), lay(l)], axis=1))


def kernel(query, key, value, q_w, q_b, k_w, k_b, v_w, v_b, scale):
    _ensure_patched_tables()
    from concourse.bass_utils import run_bass_kernel_spmd

    query = np.asarray(query, np.float32)
    key = np.asarray(key, np.float32)
    value = np.asarray(value, np.float32)

    shared = _prep_shared(
        np.asarray(q_w), np.asarray(q_b), np.asarray(k_w), np.asarray(k_b),
        np.asarray(v_w), np.asarray(v_b), np.asarray(scale),
    )

    in_maps = []
    for c in range(NCORES):
        b, h = c // 2, c % 2
        xq_full = _xT_pad8(query[b])  # [128, 2, DCH, S+2]
        xq_c = np.ascontiguousarray(xq_full[:, :, :, h * SQ : h * SQ + SQ + 2])
        m = dict(shared)
        m["xq"] = xq_c
        m["xk"] = _xT_pad8(key[b])
        m["xv"] = _xT_pad8(value[b])
        in_maps.append(m)

    nc = _get_nc()
    res = run_bass_kernel_spmd(nc, in_maps, core_ids=list(range(NCORES)))

    out_full = np.empty((B, S, D), np.float32)
    for c in range(NCORES):
        b, h = c // 2, c % 2
        out_full[b, h * SQ : (h + 1) * SQ, :] = res.results[c]["out"].reshape(SQ, D)
    return out_full


# revision 27
# speedup vs baseline: 1.0058x; 1.0000x over previous
"""Conv1d-QKV + full attention kernel for TRN2, 8 NeuronCores.

Problem (hardcoded shapes): B=4, S=4096, DIN=DQ=DK=256.
  q = conv1d(query, q_w, q_b); k = conv1d(key, ...); v = conv1d(value, ...)
  out = scale * softmax(q @ k^T / sqrt(256)) @ v

Sharding: 8 cores = (batch b = core//2) x (query half h = core%2).
Each core computes k/v conv over the full 4096 rows of its batch
(replicated across the 2 cores of a batch) and attention for its 2048
query rows.

All matmul speed comes from fp8e4m3/e5m2 DoubleRow at 0.5 cycles/row
(contracts 2x128 per instruction, 4x the f32r rate):
  - convs: hi/lo fp8 split of inputs and (64x prescaled) weights on the
    host; 3-term product xh*wh + xh*wl + xl*wh per tap (~0.15% error).
  - scores mm: q,k cast to fp8e4m3 (in 64x units) by the conv bias-add
    on DVE; DoubleRow over ci chunks. Dominant error term (~1e-2
    scale-relative; gate is 2e-2).
  - softmax weights are CENTERED before quantization: the activation
    computes expm1(s/16) via a patched act table (exp with 1.0
    subtracted from every d0 Taylor coefficient), so fp8's ~2.7%
    relative noise applies to e-1 (std ~0.37) instead of e (~1.1) --
    3x less noise. out = scale*(Sv + e_dev^T v)/(4096 + sum(e_dev)),
    where Sv = column sums of v from one extra DoubleRow chain.
  - out mm: v split into vh (e4m3) + vl (e5m2 residual); both fp8
    DoubleRow passes against e_dev. The Pool engine does the split.
Scale folds are exact powers of two: no extra ops, no extra error.
"""

import os
import json
import shutil
import struct

import numpy as np
import ml_dtypes

B, S, DIN, D = 4, 4096, 256, 256
NCORES = 8
SQ = S // 2          # query rows per core
SK = S               # key rows per core
DCH = 2              # 128-partition chunks of D / DIN
CT = 512             # conv column tile
QB = 512             # query block for attention
NBLK = SQ // QB      # 4
NKT = SK // 128      # 32 k tiles
DV = D + 4           # v_aug cols: D of v + [1,0,0,0] denominator col
VROW = 272           # v row pitch (DoubleRow group stride must be %16)
WSCALE = 64.0        # host-side prescale of conv weights/biases
EXP_SCALE = 1.0 / (16.0 * WSCALE * WSCALE)  # 1/sqrt(D) on 64x-unit scores

NP8 = ml_dtypes.float8_e4m3
NP85 = ml_dtypes.float8_e5m2

_ACT_DST = None


def _ensure_patched_tables():
    """Build an act-table dir where `exp` computes expm1, and point the
    NEFF compile at it via BASS_ACT_ROOT_JSON_PATH.

    The pwp bucket tables store per-section Taylor coefficients
    [d0,d1,d2,d3,x0] (f32 each, 32B stride). Subtracting 1.0 from every
    d0 of exp's 781 entries turns the piecewise cubic for exp into one
    for expm1 exactly (derivatives unchanged); the small-signal entry
    becomes x + x^2/2 + x^3/6 (full relative precision near 0), the
    positive saturation stays +inf, the negative one becomes -1.
    """
    global _ACT_DST
    if _ACT_DST is not None:
        return
    from neuronxcc.driver.Job import Job
    from neuronxcc.driver.jobs.support.FindActInfo import findActInfoFile

    src_json = findActInfoFile(Job.getPackageDir(), "gen3")
    src = os.path.dirname(src_json)
    dst = "/tmp/act_expm1_kernel"
    if not os.path.exists(os.path.join(dst, "act_info.json")):
        tmp = f"{dst}.tmp.{os.getpid()}"
        if os.path.exists(tmp):
            shutil.rmtree(tmp)
        shutil.copytree(src, tmp)
        bp = os.path.join(tmp, "exp_and_others_bkt.bin")
        raw = bytearray(open(bp, "rb").read())
        for i in range(781):
            (d0,) = struct.unpack_from("<f", raw, i * 32)
            struct.pack_into("<f", raw, i * 32, np.float32(d0) - np.float32(1.0))
        open(bp, "wb").write(bytes(raw))
        jp = os.path.join(tmp, "exp_and_others.json")
        sj = json.load(open(jp))
        for meta in sj["profile_meta_data"]:
            if meta["func_name"].startswith("exp"):
                meta["fzero_result"] = 0           # expm1(0) = 0
                meta["fninf_result"] = 0xBF800000  # expm1(-inf) = -1
        json.dump(sj, open(jp, "w"))
        if os.path.exists(dst):
            shutil.rmtree(dst)
        os.replace(tmp, dst)
    os.environ["BASS_ACT_ROOT_JSON_PATH"] = os.path.join(dst, "act_info.json")
    _ACT_DST = dst


def _split_drain_waits(nc):
    """Walrus in this toolchain only accepts one sem-wait per CTRL (Drain)
    instruction; Tile's kernel-tail drain carries one wait per active proc.
    Split any multi-wait Drain into a chain of single-wait drains."""
    import concourse.mybir as mybir

    def walk(blocks):
        for b in blocks:
            insts = b.instructions
            i = 0
            while i < len(insts):
                inst = insts[i]
                si = getattr(inst, "sync_info", None)
                w = list(si.on_wait) if si is not None and si.on_wait else []
                if len(w) > 1:
                    pre = [
                        mybir.InstNoOp(
                            name=f"{inst.name}-ws{j}",
                            engine=inst.engine,
                            ins=[],
                            outs=[],
                            sync_info=mybir.SyncInfo(on_wait=[wj], on_update=[]),
                        )
                        for j, wj in enumerate(w[:-1])
                    ]
                    si.on_wait = w[-1:]
                    for k, nd in enumerate(pre):
                        insts.insert(i + k, nd)
                    i += len(pre)
                i += 1
            walk(getattr(b, "blocks", []) or [])

    for f in nc.m.functions:
        walk(f.blocks)


def _build_bass():
    import concourse.bass as bass
    import concourse.mybir as mybir
    import concourse.tile as tile

    f32 = mybir.dt.float32
    fp8 = mybir.dt.float8e4
    fp8e5 = mybir.dt.float8e5
    ADD = mybir.AluOpType.add
    SUB = mybir.AluOpType.subtract
    MULT = mybir.AluOpType.mult
    DR = mybir.MatmulPerfMode.DoubleRow
    EXP = mybir.ActivationFunctionType.Exp  # table patched to expm1

    nc = bass.Bass(trn_type="TRN2")

    # ---- DRAM I/O (per-core shard shapes); dim1 = {hi, lo} fp8 pair ----
    xq = nc.dram_tensor("xq", [128, 2, DCH, SQ + 2], fp8, kind="ExternalInput")
    xk = nc.dram_tensor("xk", [128, 2, DCH, SK + 2], fp8, kind="ExternalInput")
    xv = nc.dram_tensor("xv", [128, 2, DCH, SK + 2], fp8, kind="ExternalInput")
    # [p, hl, ci_chunk a, tap t, co_chunk o, co_in_chunk m]
    wq = nc.dram_tensor("wq", [128, 2, DCH, 3, DCH, 128], fp8, kind="ExternalInput")
    wk = nc.dram_tensor("wk", [128, 2, DCH, 3, DCH, 128], fp8, kind="ExternalInput")
    # [p, hl, ci_chunk a, tap t, co]
    wv = nc.dram_tensor("wv", [128, 2, DCH, 3, D], fp8, kind="ExternalInput")
    bq = nc.dram_tensor("bq", [128, DCH], f32, kind="ExternalInput")
    bk = nc.dram_tensor("bk", [128, DCH], f32, kind="ExternalInput")
    bvb = nc.dram_tensor("bvb", [128, D], f32, kind="ExternalInput")
    scl = nc.dram_tensor("scl", [128, 1], f32, kind="ExternalInput")
    out = nc.dram_tensor("out", [SQ // 128, 128, D], f32, kind="ExternalOutput")

    TERMS = ((0, 0), (0, 1), (1, 0))  # (x hl index, w hl index): hh, hl, lh

    with tile.TileContext(nc) as tc:
        with (
            tc.tile_pool(name="persist", bufs=1) as persist,
            tc.tile_pool(name="xin", bufs=3) as xin,
            tc.tile_pool(name="epool", bufs=4) as epool,
            tc.tile_pool(name="vstg", bufs=3) as vstg,
            tc.tile_pool(name="outp", bufs=3) as outp,
            tc.tile_pool(name="tiny", bufs=4) as tiny,
            tc.tile_pool(name="ps_conv", bufs=2, space="PSUM") as ps_conv,
            tc.tile_pool(name="ps_sc", bufs=3, space="PSUM") as ps_sc,
        ):
            # ---- persistent SBUF ----
            wq_s = persist.tile([128, 2, DCH, 3, DCH, 128], fp8, tag="wq_s")
            wk_s = persist.tile([128, 2, DCH, 3, DCH, 128], fp8, tag="wk_s")
            wv_s = persist.tile([128, 2, DCH, 3, D], fp8, tag="wv_s")
            bq_s = persist.tile([128, DCH], f32, tag="bq_s")
            bk_s = persist.tile([128, DCH], f32, tag="bk_s")
            bvb_s = persist.tile([128, D], f32, tag="bvb_s")
            scl_s = persist.tile([128, 1], f32, tag="scl_s")
            qT8 = persist.tile([128, DCH, SQ], fp8, tag="qT8")
            kT8 = persist.tile([128, DCH, SK], fp8, tag="kT8")
            vh_s = persist.tile([128, NKT, VROW], fp8, tag="vh_s")
            vl_s = persist.tile([128, NKT, VROW], fp8e5, tag="vl_s")
            ones8 = persist.tile([128, 2, 128], fp8, tag="ones8")
            sv_sb = persist.tile([128, D + 1], f32, tag="sv_sb")

            # denominator column of v_aug: vh gets 1.0, vl gets 0.0
            nc.vector.memset(vh_s[:, :, D : D + 1], 1.0)
            nc.vector.memset(vl_s[:, :, D : D + 1], 0.0)
            nc.vector.memset(ones8[:], 1.0)

            def conv_T_tile(x_dram, w_s, b_s, out_s, j, tag):
                """T-layout conv col tile j: out_s[:, o, jCT:(j+1)CT] fp8."""
                xt = xin.tile([128, 2, DCH, CT + 16], fp8, tag=tag, name=tag)
                nc.sync.dma_start(
                    xt[:, :, :, 0 : CT + 2],
                    x_dram[:, :, :, j * CT : j * CT + CT + 2],
                )
                for o in range(DCH):
                    ps = ps_conv.tile([128, CT], f32, tag="ps_T", name="ps_T")
                    n = 0
                    for xi, wi in TERMS:
                        for t in range(3):
                            nc.tensor.matmul(
                                ps[:],
                                w_s[:, wi, :, t, o, :],
                                xt[:, xi, :, t : t + CT],
                                start=(n == 0),
                                stop=(n == 8),
                                perf_mode=DR,
                            )
                            n += 1
                    nc.vector.tensor_scalar(
                        out=out_s[:, o, j * CT : (j + 1) * CT],
                        in0=ps[:],
                        scalar1=b_s[:, o : o + 1],
                        scalar2=None,
                        op0=ADD,
                    )

            def conv_v_row(xt, j, rr):
                """v conv rowtile -> f32 bias-add (DVE) -> hi/lo fp8 (Pool)."""
                kt = j * (CT // 128) + rr
                ps = ps_conv.tile([128, CT], f32, tag="ps_T", name="ps_T")
                n = 0
                for xi, wi in TERMS:
                    for t in range(3):
                        nc.tensor.matmul(
                            ps[:, 0:D],
                            xt[:, xi, :, t + rr * 128 : t + rr * 128 + 128],
                            wv_s[:, wi, :, t, :],
                            start=(n == 0),
                            stop=(n == 8),
                            perf_mode=DR,
                        )
                        n += 1
                vb = vstg.tile([128, D], f32, tag="vb", name="vb")
                nc.vector.tensor_tensor(
                    out=vb[:], in0=ps[:, 0:D], in1=bvb_s[:], op=ADD
                )
                nc.gpsimd.tensor_copy(out=vh_s[:, kt, 0:D], in_=vb[:])
                nc.gpsimd.tensor_tensor(
                    out=vl_s[:, kt, 0:D], in0=vb[:], in1=vh_s[:, kt, 0:D], op=SUB
                )

            # ---- prefix: weights ride the idle Act HWDGE queue; x tiles
            # own SP from t=0 ----
            nc.scalar.dma_start(wq_s[:], wq[:])
            nc.scalar.dma_start(wk_s[:], wk[:])
            nc.scalar.dma_start(wv_s[:], wv[:])
            nc.scalar.dma_start(bvb_s[:], bvb[:])
            nc.scalar.dma_start(scl_s[:], scl[:])
            nc.sync.dma_start(bq_s[:], bq[:])
            conv_T_tile(xq, wq_s, bq_s, qT8, 0, "xt_q")
            nc.sync.dma_start(bk_s[:], bk[:])
            conv_T_tile(xk, wk_s, bk_s, kT8, 0, "xt_k")

            # ---- filler units interleaved into block 0's mm1 stream ----
            kf = [
                (lambda j=j: conv_T_tile(xk, wk_s, bk_s, kT8, j, "xt_k"))
                for j in range(1, SK // CT)
            ]
            kf.reverse()

            vxts = [None] * (SK // CT)

            def vdma(j):
                xt = xin.tile(
                    [128, 2, DCH, CT + 16], fp8, tag="xt_v", name="xt_v",
                    bufs=SK // CT,
                )
                nc.sync.dma_start(
                    xt[:, :, :, 0 : CT + 2],
                    xv[:, :, :, j * CT : j * CT + CT + 2],
                )
                vxts[j] = xt

            def vrow(r):
                conv_v_row(vxts[r // 4], r // 4, r % 4)

            e_bufs = []

            def mm1_pair(b, eb, i):
                """scores^T + expm1 for kt pair (2i, 2i+1) of query block b."""
                ps = ps_sc.tile([128, 1024], f32, tag="ps_sc", name="ps_sc")
                for h in range(2):
                    kt = 2 * i + h
                    nc.tensor.matmul(
                        ps[:, h * 512 : h * 512 + 512],
                        kT8[:, :, kt * 128 : (kt + 1) * 128],
                        qT8[:, :, b * QB : (b + 1) * QB],
                        start=True,
                        stop=True,
                        perf_mode=DR,
                    )
                nc.scalar.activation(
                    out=eb[:, 2 * i : 2 * i + 2, :],
                    in_=ps[:],
                    func=EXP,
                    scale=EXP_SCALE,
                )

            # mm2: 4 chunk-parts per qs tile, each 8 DoubleRow kt-pair matmuls
            # parts 0,1 = vh halves; parts 2,3 = vl halves (one psum group)
            states = {}

            def mm2_chunk(b, i):
                eb = e_bufs[b]
                state = states.setdefault(b, {})
                qs, part = divmod(i, 4)
                if part == 0:
                    state["po"] = ps_conv.tile(
                        [128, 512], f32, tag="ps_T", name="ps_T"
                    )
                po = state["po"]
                vt = vh_s if part < 2 else vl_s
                kp0 = 8 * (part % 2)
                for kp in range(kp0, kp0 + 8):
                    nc.tensor.matmul(
                        po[:, 0:DV],
                        eb[:, 2 * kp : 2 * kp + 2, qs * 128 : (qs + 1) * 128],
                        vt[:, 2 * kp : 2 * kp + 2, 0:DV],
                        start=(part == 0 and kp == kp0),
                        stop=(part == 3 and kp == kp0 + 7),
                        perf_mode=DR,
                    )
                if part == 3:
                    tot = outp.tile([128, D + 1], f32, tag="tot", name="tot")
                    nc.vector.tensor_tensor(
                        out=tot[:], in0=po[:, 0 : D + 1], in1=sv_sb[:], op=ADD
                    )
                    rec = tiny.tile([128, 1], f32, tag="rec", name="rec")
                    nc.vector.reciprocal(rec[:], tot[:, D : D + 1])
                    nc.vector.tensor_tensor(
                        out=rec[:], in0=rec[:], in1=scl_s[:], op=MULT
                    )
                    ot = outp.tile([128, D], f32, tag="ot", name="ot")
                    nc.vector.tensor_scalar_mul(ot[:], tot[:, 0:D], rec[:])
                    nc.sync.dma_start(out[b * (QB // 128) + qs, :, :], ot[:])

            def emit_sv():
                sv_ps = ps_conv.tile([128, CT], f32, tag="ps_T", name="ps_T")
                n = 0
                for vt in (vh_s, vl_s):
                    for kp in range(NKT // 2):
                        nc.tensor.matmul(
                            sv_ps[:, 0:DV],
                            ones8[:],
                            vt[:, 2 * kp : 2 * kp + 2, 0:DV],
                            start=(n == 0),
                            stop=(n == NKT - 1),
                            perf_mode=DR,
                        )
                        n += 1
                nc.vector.tensor_copy(out=sv_sb[:], in_=sv_ps[:, 0 : D + 1])

            # ---- attention pipeline ----
            # window 0: mm1(0) paced against k-conv tiles; v DMAs prefetch;
            # q-conv tile b+1 lands at the end of window b (before the
            # pairs of window b+1 that read it)
            eb0 = epool.tile([128, NKT, QB], fp8, tag="e_s", name="e_s")
            e_bufs.append(eb0)
            for i in range(NKT // 2):
                mm1_pair(0, eb0, i)
                if kf:
                    kf.pop()()
                if i >= 8:
                    vdma(i - 8)
            conv_T_tile(xq, wq_s, bq_s, qT8, 1, "xt_q")

            # per-pair unit schedules for windows 1..3 (PE work under the
            # ~17.5us exp budget per window); mm2(b) runs 2-3 windows after
            # its exp so v-conv can fill window 1
            def U_vrow(r):
                return lambda: vrow(r)

            def U_chunk(b, j):
                return lambda: mm2_chunk(b, j)

            UNITS = {1: [[] for _ in range(16)], 2: [[] for _ in range(16)],
                     3: [[] for _ in range(16)]}
            for r in range(25):          # w1: v rows 0..24, 1-2 per pair
                UNITS[1][(r * 16) // 25].append(U_vrow(r))
            for r in range(25, 32):      # w2: v rows 25..31 on pairs 0..3
                UNITS[2][(r - 25) // 2].append(U_vrow(r))
            UNITS[2][4].append(emit_sv)
            for n, (bb, j) in enumerate([(0, j) for j in range(16)]):
                UNITS[2][5 + (n * 11) // 16].append(U_chunk(bb, j))
            W3 = [(1, j) for j in range(16)] + [(2, j) for j in range(16)]
            for n, (bb, j) in enumerate(W3):
                UNITS[3][(n * 16) // len(W3)].append(U_chunk(bb, j))


            for b in range(1, NBLK):
                eb = epool.tile([128, NKT, QB], fp8, tag="e_s", name="e_s")
                e_bufs.append(eb)
                for i in range(NKT // 2):
                    mm1_pair(b, eb, i)
                    for u in UNITS[b][i]:
                        u()
                if b < NBLK - 1:
                    conv_T_tile(xq, wq_s, bq_s, qT8, b + 1, "xt_q")
            for bb, j in [(3, j) for j in range(16)]:
                mm2_chunk(bb, j)

    _split_drain_waits(nc)
    return nc


_NC_CACHE = None


def _get_nc():
    global _NC_CACHE
    if _NC_CACHE is None:
        _NC_CACHE = _build_bass()
    return _NC_CACHE


def _split8(a):
    h = a.astype(NP8)
    l = (a - h.astype(np.float32)).astype(NP8)
    return h, l


def _prep_shared(q_w, q_b, k_w, k_b, v_w, v_b, scale):
    def w_T(w):  # [co, ci, 3] -> [p, hl, a, t, o, m] fp8 of 64*w
        arr = np.ascontiguousarray((w * WSCALE).transpose(1, 2, 0)).astype(np.float32)
        arr = arr.reshape(DCH, 128, 3, DCH, 128).transpose(1, 0, 2, 3, 4)
        h, l = _split8(arr)
        return np.ascontiguousarray(np.stack([h, l], axis=1))

    def w_v(w):  # [co, ci, 3] -> [p, hl, a, t, co] fp8 of 64*w
        arr = np.ascontiguousarray((w * WSCALE).transpose(1, 2, 0)).astype(np.float32)
        arr = arr.reshape(DCH, 128, 3, D).transpose(1, 0, 2, 3)
        h, l = _split8(arr)
        return np.ascontiguousarray(np.stack([h, l], axis=1))

    def b_T(b):  # [co] -> [p, o] of 64*b
        return np.ascontiguousarray(
            (np.asarray(b, np.float64) * WSCALE).astype(np.float32).reshape(DCH, 128).T
        )

    return {
        "wq": w_T(q_w),
        "wk": w_T(k_w),
        "wv": w_v(v_w),
        "bq": b_T(q_b),
        "bk": b_T(k_b),
        "bvb": np.ascontiguousarray(
            np.tile((np.asarray(v_b, np.float32) * np.float32(WSCALE)), (128, 1))
        ),
        "scl": np.full((128, 1), float(scale) / WSCALE, np.float32),
    }


def _xT_pad8(x_b):
    """[S, C] -> transposed+halo-padded hi/lo fp8 [128, 2, DCH, S+2]."""
    xt = np.zeros((DIN, x_b.shape[0] + 2), np.float32)
    xt[:, 1:-1] = x_b.T
    h, l = _split8(xt)

    def lay(a):
        return a.reshape(DCH, 128, -1).transpose(1, 0, 2)

    return np.ascontiguousarray(np.stack([lay(h), lay(l)], axis=1))


def kernel(query, key, value, q_w, q_b, k_w, k_b, v_w, v_b, scale):
    _ensure_patched_tables()
    from concourse.bass_utils import run_bass_kernel_spmd

    query = np.asarray(query, np.float32)
    key = np.asarray(key, np.float32)
    value = np.asarray(value, np.float32)

    shared = _prep_shared(
        np.asarray(q_w), np.asarray(q_b), np.asarray(k_w), np.asarray(k_b),
        np.asarray(v_w), np.asarray(v_b), np.asarray(scale),
    )

    in_maps = []
    for c in range(NCORES):
        b, h = c // 2, c % 2
        xq_full = _xT_pad8(query[b])  # [128, 2, DCH, S+2]
        xq_c = np.ascontiguousarray(xq_full[:, :, :, h * SQ : h * SQ + SQ + 2])
        m = dict(shared)
        m["xq"] = xq_c
        m["xk"] = _xT_pad8(key[b])
        m["xv"] = _xT_pad8(value[b])
        in_maps.append(m)

    nc = _get_nc()
    res = run_bass_kernel_spmd(nc, in_maps, core_ids=list(range(NCORES)))

    out_full = np.empty((B, S, D), np.float32)
    for c in range(NCORES):
        b, h = c // 2, c % 2
        out_full[b, h * SQ : (h + 1) * SQ, :] = res.results[c]["out"].reshape(SQ, D)
    return out_full


# revision 28
# speedup vs baseline: 1.0708x; 1.0646x over previous
"""Conv1d-QKV + full attention kernel for TRN2, 8 NeuronCores.

Problem (hardcoded shapes): B=4, S=4096, DIN=DQ=DK=256.
  q = conv1d(query, q_w, q_b); k = conv1d(key, ...); v = conv1d(value, ...)
  out = scale * softmax(q @ k^T / sqrt(256)) @ v

Sharding: 8 cores = (batch b = core//2) x (query half h = core%2).
Each core computes k/v conv over the full 4096 rows of its batch
(replicated across the 2 cores of a batch) and attention for its 2048
query rows.

All matmul speed comes from fp8e4m3/e5m2 DoubleRow at 0.5 cycles/row
(contracts 2x128 per instruction, 4x the f32r rate):
  - convs: hi/lo fp8 split of inputs and (64x prescaled) weights on the
    host; 3-term product xh*wh + xh*wl + xl*wh per tap (~0.15% error).
  - scores mm: q,k cast to fp8e4m3 (in 64x units) by the conv bias-add
    on DVE; DoubleRow over ci chunks. Dominant error term (~1e-2
    scale-relative; gate is 2e-2).
  - softmax weights are CENTERED before quantization: the activation
    computes expm1(s/16) via a patched act table (exp with 1.0
    subtracted from every d0 Taylor coefficient), so fp8's ~2.7%
    relative noise applies to e-1 (std ~0.37) instead of e (~1.1) --
    3x less noise. out = scale*(Sv + e_dev^T v)/(4096 + sum(e_dev)),
    where Sv = column sums of v from one extra DoubleRow chain.
  - out mm: v split into vh (e4m3) + vl (e5m2 residual); both fp8
    DoubleRow passes against e_dev. The Pool engine does the split.
Scale folds are exact powers of two: no extra ops, no extra error.
"""

import os
import json
import shutil
import struct

import numpy as np
import ml_dtypes

B, S, DIN, D = 4, 4096, 256, 256
NCORES = 8
SQ = S // 2          # query rows per core
SK = S               # key rows per core
DCH = 2              # 128-partition chunks of D / DIN
CT = 512             # conv column tile
QB = 512             # query block for attention
NBLK = SQ // QB      # 4
NKT = SK // 128      # 32 k tiles
DV = D + 4           # v_aug cols: D of v + [1,0,0,0] denominator col
VROW = 272           # v row pitch (DoubleRow group stride must be %16)
WSCALE = 64.0        # host-side prescale of conv weights/biases
EXP_SCALE = 1.0 / (16.0 * WSCALE * WSCALE)  # 1/sqrt(D) on 64x-unit scores

NP8 = ml_dtypes.float8_e4m3
NP85 = ml_dtypes.float8_e5m2

_ACT_DST = None


def _ensure_patched_tables():
    """Build an act-table dir where `exp` computes expm1, and point the
    NEFF compile at it via BASS_ACT_ROOT_JSON_PATH.

    The pwp bucket tables store per-section Taylor coefficients
    [d0,d1,d2,d3,x0] (f32 each, 32B stride). Subtracting 1.0 from every
    d0 of exp's 781 entries turns the piecewise cubic for exp into one
    for expm1 exactly (derivatives unchanged); the small-signal entry
    becomes x + x^2/2 + x^3/6 (full relative precision near 0), the
    positive saturation stays +inf, the negative one becomes -1.
    """
    global _ACT_DST
    if _ACT_DST is not None:
        return
    from neuronxcc.driver.Job import Job
    from neuronxcc.driver.jobs.support.FindActInfo import findActInfoFile

    src_json = findActInfoFile(Job.getPackageDir(), "gen3")
    src = os.path.dirname(src_json)
    dst = "/tmp/act_expm1_kernel"
    if not os.path.exists(os.path.join(dst, "act_info.json")):
        tmp = f"{dst}.tmp.{os.getpid()}"
        if os.path.exists(tmp):
            shutil.rmtree(tmp)
        shutil.copytree(src, tmp)
        bp = os.path.join(tmp, "exp_and_others_bkt.bin")
        raw = bytearray(open(bp, "rb").read())
        for i in range(781):
            (d0,) = struct.unpack_from("<f", raw, i * 32)
            struct.pack_into("<f", raw, i * 32, np.float32(d0) - np.float32(1.0))
        open(bp, "wb").write(bytes(raw))
        jp = os.path.join(tmp, "exp_and_others.json")
        sj = json.load(open(jp))
        for meta in sj["profile_meta_data"]:
            if meta["func_name"].startswith("exp"):
                meta["fzero_result"] = 0           # expm1(0) = 0
                meta["fninf_result"] = 0xBF800000  # expm1(-inf) = -1
        json.dump(sj, open(jp, "w"))
        if os.path.exists(dst):
            shutil.rmtree(dst)
        os.replace(tmp, dst)
    os.environ["BASS_ACT_ROOT_JSON_PATH"] = os.path.join(dst, "act_info.json")
    _ACT_DST = dst


def _split_drain_waits(nc):
    """Walrus in this toolchain only accepts one sem-wait per CTRL (Drain)
    instruction; Tile's kernel-tail drain carries one wait per active proc.
    Split any multi-wait Drain into a chain of single-wait drains."""
    import concourse.mybir as mybir

    def walk(blocks):
        for b in blocks:
            insts = b.instructions
            i = 0
            while i < len(insts):
                inst = insts[i]
                si = getattr(inst, "sync_info", None)
                w = list(si.on_wait) if si is not None and si.on_wait else []
                if len(w) > 1:
                    pre = [
                        mybir.InstNoOp(
                            name=f"{inst.name}-ws{j}",
                            engine=inst.engine,
                            ins=[],
                            outs=[],
                            sync_info=mybir.SyncInfo(on_wait=[wj], on_update=[]),
                        )
                        for j, wj in enumerate(w[:-1])
                    ]
                    si.on_wait = w[-1:]
                    for k, nd in enumerate(pre):
                        insts.insert(i + k, nd)
                    i += len(pre)
                i += 1
            walk(getattr(b, "blocks", []) or [])

    for f in nc.m.functions:
        walk(f.blocks)


def _build_bass():
    import concourse.bass as bass
    import concourse.mybir as mybir
    import concourse.tile as tile

    f32 = mybir.dt.float32
    fp8 = mybir.dt.float8e4
    fp8e5 = mybir.dt.float8e5
    ADD = mybir.AluOpType.add
    SUB = mybir.AluOpType.subtract
    MULT = mybir.AluOpType.mult
    DR = mybir.MatmulPerfMode.DoubleRow
    EXP = mybir.ActivationFunctionType.Exp  # table patched to expm1

    nc = bass.Bass(trn_type="TRN2")

    # ---- DRAM I/O (per-core shard shapes); dim1 = {hi, lo} fp8 pair ----
    xq = nc.dram_tensor("xq", [128, 2, DCH, SQ + 2], fp8, kind="ExternalInput")
    xk = nc.dram_tensor("xk", [128, 2, DCH, SK + 2], fp8, kind="ExternalInput")
    xv = nc.dram_tensor("xv", [128, 2, DCH, SK + 2], fp8, kind="ExternalInput")
    # [p, hl, ci_chunk a, tap t, co_chunk o, co_in_chunk m]
    wq = nc.dram_tensor("wq", [128, 2, DCH, 3, DCH, 128], fp8, kind="ExternalInput")
    wk = nc.dram_tensor("wk", [128, 2, DCH, 3, DCH, 128], fp8, kind="ExternalInput")
    # [p, hl, ci_chunk a, tap t, co]
    wv = nc.dram_tensor("wv", [128, 2, DCH, 3, D], fp8, kind="ExternalInput")
    bq = nc.dram_tensor("bq", [128, DCH], f32, kind="ExternalInput")
    bk = nc.dram_tensor("bk", [128, DCH], f32, kind="ExternalInput")
    bvb = nc.dram_tensor("bvb", [128, D], f32, kind="ExternalInput")
    scl = nc.dram_tensor("scl", [128, 1], f32, kind="ExternalInput")
    out = nc.dram_tensor("out", [SQ // 128, 128, D], f32, kind="ExternalOutput")

    TERMS = ((0, 0), (0, 1), (1, 0))  # (x hl index, w hl index): hh, hl, lh

    with tile.TileContext(nc) as tc:
        with (
            tc.tile_pool(name="persist", bufs=1) as persist,
            tc.tile_pool(name="xin", bufs=3) as xin,
            tc.tile_pool(name="epool", bufs=4) as epool,
            tc.tile_pool(name="vstg", bufs=3) as vstg,
            tc.tile_pool(name="outp", bufs=3) as outp,
            tc.tile_pool(name="tiny", bufs=4) as tiny,
            tc.tile_pool(name="ps_conv", bufs=2, space="PSUM") as ps_conv,
            tc.tile_pool(name="ps_sc", bufs=3, space="PSUM") as ps_sc,
        ):
            # ---- persistent SBUF ----
            wq_s = persist.tile([128, 2, DCH, 3, DCH, 128], fp8, tag="wq_s")
            wk_s = persist.tile([128, 2, DCH, 3, DCH, 128], fp8, tag="wk_s")
            wv_s = persist.tile([128, 2, DCH, 3, D], fp8, tag="wv_s")
            bq_s = persist.tile([128, DCH], f32, tag="bq_s")
            bk_s = persist.tile([128, DCH], f32, tag="bk_s")
            bvb_s = persist.tile([128, D], f32, tag="bvb_s")
            scl_s = persist.tile([128, 1], f32, tag="scl_s")
            qT8 = persist.tile([128, DCH, SQ], fp8, tag="qT8")
            kT8 = persist.tile([128, DCH, SK], fp8, tag="kT8")
            vh_s = persist.tile([128, NKT, VROW], fp8, tag="vh_s")
            vl_s = persist.tile([128, NKT, VROW], fp8e5, tag="vl_s")
            ones8 = persist.tile([128, 2, 128], fp8, tag="ones8")
            sv_sb = persist.tile([128, D + 1], f32, tag="sv_sb")

            # denominator column of v_aug: vh gets 1.0, vl gets 0.0
            nc.vector.memset(vh_s[:, :, D : D + 1], 1.0)
            nc.vector.memset(vl_s[:, :, D : D + 1], 0.0)
            nc.vector.memset(ones8[:], 1.0)

            # q/k T-convs use 2 terms (xh+xl)*wh: the dropped w-residual
            # (~2.7% conv noise) adds in quadrature to the unavoidable fp8
            # cast of q/k, raising the mm1 error term ~1.4x (still in gate)
            QK_TERMS = ((0, 0), (1, 0))

            def conv_T_tile(x_dram, w_s, b_s, out_s, j, tag):
                """T-layout conv col tile j: out_s[:, o, jCT:(j+1)CT] fp8."""
                xt = xin.tile([128, 2, DCH, CT + 16], fp8, tag=tag, name=tag)
                nc.sync.dma_start(
                    xt[:, :, :, 0 : CT + 2],
                    x_dram[:, :, :, j * CT : j * CT + CT + 2],
                )
                last = 3 * len(QK_TERMS) - 1
                for o in range(DCH):
                    ps = ps_conv.tile([128, CT], f32, tag="ps_T", name="ps_T")
                    n = 0
                    for xi, wi in QK_TERMS:
                        for t in range(3):
                            nc.tensor.matmul(
                                ps[:],
                                w_s[:, wi, :, t, o, :],
                                xt[:, xi, :, t : t + CT],
                                start=(n == 0),
                                stop=(n == last),
                                perf_mode=DR,
                            )
                            n += 1
                    nc.vector.tensor_scalar(
                        out=out_s[:, o, j * CT : (j + 1) * CT],
                        in0=ps[:],
                        scalar1=b_s[:, o : o + 1],
                        scalar2=None,
                        op0=ADD,
                    )

            def conv_v_row(xt, j, rr):
                """v conv rowtile -> f32 bias-add (DVE) -> hi/lo fp8 (Pool)."""
                kt = j * (CT // 128) + rr
                ps = ps_conv.tile([128, CT], f32, tag="ps_T", name="ps_T")
                n = 0
                for xi, wi in TERMS:
                    for t in range(3):
                        nc.tensor.matmul(
                            ps[:, 0:D],
                            xt[:, xi, :, t + rr * 128 : t + rr * 128 + 128],
                            wv_s[:, wi, :, t, :],
                            start=(n == 0),
                            stop=(n == 8),
                            perf_mode=DR,
                        )
                        n += 1
                vb = vstg.tile([128, D], f32, tag="vb", name="vb")
                nc.vector.tensor_tensor(
                    out=vb[:], in0=ps[:, 0:D], in1=bvb_s[:], op=ADD
                )
                nc.gpsimd.tensor_copy(out=vh_s[:, kt, 0:D], in_=vb[:])
                nc.gpsimd.tensor_tensor(
                    out=vl_s[:, kt, 0:D], in0=vb[:], in1=vh_s[:, kt, 0:D], op=SUB
                )

            # ---- prefix: weights ride the idle Act HWDGE queue; x tiles
            # own SP from t=0 ----
            nc.scalar.dma_start(wq_s[:], wq[:])
            nc.scalar.dma_start(wk_s[:], wk[:])
            nc.scalar.dma_start(wv_s[:], wv[:])
            nc.scalar.dma_start(bvb_s[:], bvb[:])
            nc.scalar.dma_start(scl_s[:], scl[:])
            nc.sync.dma_start(bq_s[:], bq[:])
            conv_T_tile(xq, wq_s, bq_s, qT8, 0, "xt_q")
            nc.sync.dma_start(bk_s[:], bk[:])
            conv_T_tile(xk, wk_s, bk_s, kT8, 0, "xt_k")

            # ---- filler units interleaved into block 0's mm1 stream ----
            kf = [
                (lambda j=j: conv_T_tile(xk, wk_s, bk_s, kT8, j, "xt_k"))
                for j in range(1, SK // CT)
            ]
            kf.reverse()

            vxts = [None] * (SK // CT)

            def vdma(j):
                xt = xin.tile(
                    [128, 2, DCH, CT + 16], fp8, tag="xt_v", name="xt_v",
                    bufs=SK // CT,
                )
                nc.sync.dma_start(
                    xt[:, :, :, 0 : CT + 2],
                    xv[:, :, :, j * CT : j * CT + CT + 2],
                )
                vxts[j] = xt

            def vrow(r):
                conv_v_row(vxts[r // 4], r // 4, r % 4)

            e_bufs = []

            def mm1_pair(b, eb, i):
                """scores^T + expm1 for kt pair (2i, 2i+1) of query block b."""
                ps = ps_sc.tile([128, 1024], f32, tag="ps_sc", name="ps_sc")
                for h in range(2):
                    kt = 2 * i + h
                    nc.tensor.matmul(
                        ps[:, h * 512 : h * 512 + 512],
                        kT8[:, :, kt * 128 : (kt + 1) * 128],
                        qT8[:, :, b * QB : (b + 1) * QB],
                        start=True,
                        stop=True,
                        perf_mode=DR,
                    )
                nc.scalar.activation(
                    out=eb[:, 2 * i : 2 * i + 2, :],
                    in_=ps[:],
                    func=EXP,
                    scale=EXP_SCALE,
                )

            # mm2: 4 chunk-parts per qs tile, each 8 DoubleRow kt-pair matmuls
            # parts 0,1 = vh halves; parts 2,3 = vl halves (one psum group)
            states = {}

            def mm2_chunk(b, i):
                eb = e_bufs[b]
                state = states.setdefault(b, {})
                qs, part = divmod(i, 4)
                if part == 0:
                    state["po"] = ps_conv.tile(
                        [128, 512], f32, tag="ps_T", name="ps_T"
                    )
                po = state["po"]
                vt = vh_s if part < 2 else vl_s
                kp0 = 8 * (part % 2)
                for kp in range(kp0, kp0 + 8):
                    nc.tensor.matmul(
                        po[:, 0:DV],
                        eb[:, 2 * kp : 2 * kp + 2, qs * 128 : (qs + 1) * 128],
                        vt[:, 2 * kp : 2 * kp + 2, 0:DV],
                        start=(part == 0 and kp == kp0),
                        stop=(part == 3 and kp == kp0 + 7),
                        perf_mode=DR,
                    )
                if part == 3:
                    tot = outp.tile([128, D + 1], f32, tag="tot", name="tot")
                    nc.vector.tensor_tensor(
                        out=tot[:], in0=po[:, 0 : D + 1], in1=sv_sb[:], op=ADD
                    )
                    rec = tiny.tile([128, 1], f32, tag="rec", name="rec")
                    nc.vector.reciprocal(rec[:], tot[:, D : D + 1])
                    nc.vector.tensor_tensor(
                        out=rec[:], in0=rec[:], in1=scl_s[:], op=MULT
                    )
                    ot = outp.tile([128, D], f32, tag="ot", name="ot")
                    nc.vector.tensor_scalar_mul(ot[:], tot[:, 0:D], rec[:])
                    nc.sync.dma_start(out[b * (QB // 128) + qs, :, :], ot[:])

            def emit_sv():
                sv_ps = ps_conv.tile([128, CT], f32, tag="ps_T", name="ps_T")
                n = 0
                for vt in (vh_s, vl_s):
                    for kp in range(NKT // 2):
                        nc.tensor.matmul(
                            sv_ps[:, 0:DV],
                            ones8[:],
                            vt[:, 2 * kp : 2 * kp + 2, 0:DV],
                            start=(n == 0),
                            stop=(n == NKT - 1),
                            perf_mode=DR,
                        )
                        n += 1
                nc.vector.tensor_copy(out=sv_sb[:], in_=sv_ps[:, 0 : D + 1])

            # ---- attention pipeline ----
            # window 0: mm1(0) paced against k-conv tiles; v DMAs prefetch;
            # q-conv tile b+1 lands at the end of window b (before the
            # pairs of window b+1 that read it)
            eb0 = epool.tile([128, NKT, QB], fp8, tag="e_s", name="e_s")
            e_bufs.append(eb0)
            for i in range(NKT // 2):
                mm1_pair(0, eb0, i)
                if kf:
                    kf.pop()()
                if i >= 8:
                    vdma(i - 8)
            conv_T_tile(xq, wq_s, bq_s, qT8, 1, "xt_q")

            # per-pair unit schedules for windows 1..3 (PE work under the
            # ~17.5us exp budget per window); mm2(b) runs 2-3 windows after
            # its exp so v-conv can fill window 1
            def U_vrow(r):
                return lambda: vrow(r)

            def U_chunk(b, j):
                return lambda: mm2_chunk(b, j)

            UNITS = {1: [[] for _ in range(16)], 2: [[] for _ in range(16)],
                     3: [[] for _ in range(16)]}
            for r in range(25):          # w1: v rows 0..24, 1-2 per pair
                UNITS[1][(r * 16) // 25].append(U_vrow(r))
            for r in range(25, 32):      # w2: v rows 25..31 on pairs 0..3
                UNITS[2][(r - 25) // 2].append(U_vrow(r))
            UNITS[2][4].append(emit_sv)
            for n, (bb, j) in enumerate([(0, j) for j in range(16)]):
                UNITS[2][5 + (n * 11) // 16].append(U_chunk(bb, j))
            W3 = [(1, j) for j in range(16)] + [(2, j) for j in range(16)]
            for n, (bb, j) in enumerate(W3):
                UNITS[3][(n * 16) // len(W3)].append(U_chunk(bb, j))


            for b in range(1, NBLK):
                eb = epool.tile([128, NKT, QB], fp8, tag="e_s", name="e_s")
                e_bufs.append(eb)
                for i in range(NKT // 2):
                    mm1_pair(b, eb, i)
                    for u in UNITS[b][i]:
                        u()
                if b < NBLK - 1:
                    conv_T_tile(xq, wq_s, bq_s, qT8, b + 1, "xt_q")
            for bb, j in [(3, j) for j in range(16)]:
                mm2_chunk(bb, j)

    _split_drain_waits(nc)
    return nc


_NC_CACHE = None


def _get_nc():
    global _NC_CACHE
    if _NC_CACHE is None:
        _NC_CACHE = _build_bass()
    return _NC_CACHE


def _split8(a):
    h = a.astype(NP8)
    l = (a - h.astype(np.float32)).astype(NP8)
    return h, l


def _prep_shared(q_w, q_b, k_w, k_b, v_w, v_b, scale):
    def w_T(w):  # [co, ci, 3] -> [p, hl, a, t, o, m] fp8 of 64*w
        arr = np.ascontiguousarray((w * WSCALE).transpose(1, 2, 0)).astype(np.float32)
        arr = arr.reshape(DCH, 128, 3, DCH, 128).transpose(1, 0, 2, 3, 4)
        h, l = _split8(arr)
        return np.ascontiguousarray(np.stack([h, l], axis=1))

    def w_v(w):  # [co, ci, 3] -> [p, hl, a, t, co] fp8 of 64*w
        arr = np.ascontiguousarray((w * WSCALE).transpose(1, 2, 0)).astype(np.float32)
        arr = arr.reshape(DCH, 128, 3, D).transpose(1, 0, 2, 3)
        h, l = _split8(arr)
        return np.ascontiguousarray(np.stack([h, l], axis=1))

    def b_T(b):  # [co] -> [p, o] of 64*b
        return np.ascontiguousarray(
            (np.asarray(b, np.float64) * WSCALE).astype(np.float32).reshape(DCH, 128).T
        )

    return {
        "wq": w_T(q_w),
        "wk": w_T(k_w),
        "wv": w_v(v_w),
        "bq": b_T(q_b),
        "bk": b_T(k_b),
        "bvb": np.ascontiguousarray(
            np.tile((np.asarray(v_b, np.float32) * np.float32(WSCALE)), (128, 1))
        ),
        "scl": np.full((128, 1), float(scale) / WSCALE, np.float32),
    }


def _xT_pad8(x_b):
    """[S, C] -> transposed+halo-padded hi/lo fp8 [128, 2, DCH, S+2]."""
    xt = np.zeros((DIN, x_b.shape[0] + 2), np.float32)
    xt[:, 1:-1] = x_b.T
    h, l = _split8(xt)

    def lay(a):
        return a.reshape(DCH, 128, -1).transpose(1, 0, 2)

    return np.ascontiguousarray(np.stack([lay(h), lay(l)], axis=1))


def kernel(query, key, value, q_w, q_b, k_w, k_b, v_w, v_b, scale):
    _ensure_patched_tables()
    from concourse.bass_utils import run_bass_kernel_spmd

    query = np.asarray(query, np.float32)
    key = np.asarray(key, np.float32)
    value = np.asarray(value, np.float32)

    shared = _prep_shared(
        np.asarray(q_w), np.asarray(q_b), np.asarray(k_w), np.asarray(k_b),
        np.asarray(v_w), np.asarray(v_b), np.asarray(scale),
    )

    in_maps = []
    for c in range(NCORES):
        b, h = c // 2, c % 2
        xq_full = _xT_pad8(query[b])  # [128, 2, DCH, S+2]
        xq_c = np.ascontiguousarray(xq_full[:, :, :, h * SQ : h * SQ + SQ + 2])
        m = dict(shared)
        m["xq"] = xq_c
        m["xk"] = _xT_pad8(key[b])
        m["xv"] = _xT_pad8(value[b])
        in_maps.append(m)

    nc = _get_nc()
    res = run_bass_kernel_spmd(nc, in_maps, core_ids=list(range(NCORES)))

    out_full = np.empty((B, S, D), np.float32)
    for c in range(NCORES):
        b, h = c // 2, c % 2
        out_full[b, h * SQ : (h + 1) * SQ, :] = res.results[c]["out"].reshape(SQ, D)
    return out_full


# revision 29
# speedup vs baseline: 1.0740x; 1.0029x over previous
"""Conv1d-QKV + full attention kernel for TRN2, 8 NeuronCores.

Problem (hardcoded shapes): B=4, S=4096, DIN=DQ=DK=256.
  q = conv1d(query, q_w, q_b); k = conv1d(key, ...); v = conv1d(value, ...)
  out = scale * softmax(q @ k^T / sqrt(256)) @ v

Sharding: 8 cores = (batch b = core//2) x (query half h = core%2).
Each core computes k/v conv over the full 4096 rows of its batch
(replicated across the 2 cores of a batch) and attention for its 2048
query rows.

All matmul speed comes from fp8e4m3/e5m2 DoubleRow at 0.5 cycles/row
(contracts 2x128 per instruction, 4x the f32r rate):
  - convs: hi/lo fp8 split of inputs and (64x prescaled) weights on the
    host; 3-term product xh*wh + xh*wl + xl*wh per tap (~0.15% error).
  - scores mm: q,k cast to fp8e4m3 (in 64x units) by the conv bias-add
    on DVE; DoubleRow over ci chunks. Dominant error term (~1e-2
    scale-relative; gate is 2e-2).
  - softmax weights are CENTERED before quantization: the activation
    computes expm1(s/16) via a patched act table (exp with 1.0
    subtracted from every d0 Taylor coefficient), so fp8's ~2.7%
    relative noise applies to e-1 (std ~0.37) instead of e (~1.1) --
    3x less noise. out = scale*(Sv + e_dev^T v)/(4096 + sum(e_dev)),
    where Sv = column sums of v from one extra DoubleRow chain.
  - out mm: v split into vh (e4m3) + vl (e5m2 residual); both fp8
    DoubleRow passes against e_dev. The Pool engine does the split.
Scale folds are exact powers of two: no extra ops, no extra error.
"""

import os
import json
import shutil
import struct

import numpy as np
import ml_dtypes

B, S, DIN, D = 4, 4096, 256, 256
NCORES = 8
SQ = S // 2          # query rows per core
SK = S               # key rows per core
DCH = 2              # 128-partition chunks of D / DIN
CT = 512             # conv column tile
QB = 512             # query block for attention
NBLK = SQ // QB      # 4
NKT = SK // 128      # 32 k tiles
DV = D + 4           # v_aug cols: D of v + [1,0,0,0] denominator col
VROW = 272           # v row pitch (DoubleRow group stride must be %16)
WSCALE = 64.0        # host-side prescale of conv weights/biases
EXP_SCALE = 1.0 / (16.0 * WSCALE * WSCALE)  # 1/sqrt(D) on 64x-unit scores

NP8 = ml_dtypes.float8_e4m3
NP85 = ml_dtypes.float8_e5m2

_ACT_DST = None


def _ensure_patched_tables():
    """Build an act-table dir where `exp` computes expm1, and point the
    NEFF compile at it via BASS_ACT_ROOT_JSON_PATH.

    The pwp bucket tables store per-section Taylor coefficients
    [d0,d1,d2,d3,x0] (f32 each, 32B stride). Subtracting 1.0 from every
    d0 of exp's 781 entries turns the piecewise cubic for exp into one
    for expm1 exactly (derivatives unchanged); the small-signal entry
    becomes x + x^2/2 + x^3/6 (full relative precision near 0), the
    positive saturation stays +inf, the negative one becomes -1.
    """
    global _ACT_DST
    if _ACT_DST is not None:
        return
    from neuronxcc.driver.Job import Job
    from neuronxcc.driver.jobs.support.FindActInfo import findActInfoFile

    src_json = findActInfoFile(Job.getPackageDir(), "gen3")
    src = os.path.dirname(src_json)
    dst = "/tmp/act_expm1_kernel"
    if not os.path.exists(os.path.join(dst, "act_info.json")):
        tmp = f"{dst}.tmp.{os.getpid()}"
        if os.path.exists(tmp):
            shutil.rmtree(tmp)
        shutil.copytree(src, tmp)
        bp = os.path.join(tmp, "exp_and_others_bkt.bin")
        raw = bytearray(open(bp, "rb").read())
        for i in range(781):
            (d0,) = struct.unpack_from("<f", raw, i * 32)
            struct.pack_into("<f", raw, i * 32, np.float32(d0) - np.float32(1.0))
        open(bp, "wb").write(bytes(raw))
        jp = os.path.join(tmp, "exp_and_others.json")
        sj = json.load(open(jp))
        for meta in sj["profile_meta_data"]:
            if meta["func_name"].startswith("exp"):
                meta["fzero_result"] = 0           # expm1(0) = 0
                meta["fninf_result"] = 0xBF800000  # expm1(-inf) = -1
        json.dump(sj, open(jp, "w"))
        if os.path.exists(dst):
            shutil.rmtree(dst)
        os.replace(tmp, dst)
    os.environ["BASS_ACT_ROOT_JSON_PATH"] = os.path.join(dst, "act_info.json")
    _ACT_DST = dst


def _split_drain_waits(nc):
    """Walrus in this toolchain only accepts one sem-wait per CTRL (Drain)
    instruction; Tile's kernel-tail drain carries one wait per active proc.
    Split any multi-wait Drain into a chain of single-wait drains."""
    import concourse.mybir as mybir

    def walk(blocks):
        for b in blocks:
            insts = b.instructions
            i = 0
            while i < len(insts):
                inst = insts[i]
                si = getattr(inst, "sync_info", None)
                w = list(si.on_wait) if si is not None and si.on_wait else []
                if len(w) > 1:
                    pre = [
                        mybir.InstNoOp(
                            name=f"{inst.name}-ws{j}",
                            engine=inst.engine,
                            ins=[],
                            outs=[],
                            sync_info=mybir.SyncInfo(on_wait=[wj], on_update=[]),
                        )
                        for j, wj in enumerate(w[:-1])
                    ]
                    si.on_wait = w[-1:]
                    for k, nd in enumerate(pre):
                        insts.insert(i + k, nd)
                    i += len(pre)
                i += 1
            walk(getattr(b, "blocks", []) or [])

    for f in nc.m.functions:
        walk(f.blocks)


def _build_bass():
    import concourse.bass as bass
    import concourse.mybir as mybir
    import concourse.tile as tile

    f32 = mybir.dt.float32
    fp8 = mybir.dt.float8e4
    fp8e5 = mybir.dt.float8e5
    ADD = mybir.AluOpType.add
    SUB = mybir.AluOpType.subtract
    MULT = mybir.AluOpType.mult
    DR = mybir.MatmulPerfMode.DoubleRow
    EXP = mybir.ActivationFunctionType.Exp  # table patched to expm1

    nc = bass.Bass(trn_type="TRN2")

    # ---- DRAM I/O (per-core shard shapes); dim1 = {hi, lo} fp8 pair ----
    xq = nc.dram_tensor("xq", [128, 2, DCH, SQ + 2], fp8, kind="ExternalInput")
    xk = nc.dram_tensor("xk", [128, 2, DCH, SK + 2], fp8, kind="ExternalInput")
    xv = nc.dram_tensor("xv", [128, 2, DCH, SK + 2], fp8, kind="ExternalInput")
    # [p, hl, ci_chunk a, tap t, co_chunk o, co_in_chunk m]
    wq = nc.dram_tensor("wq", [128, 2, DCH, 3, DCH, 128], fp8, kind="ExternalInput")
    wk = nc.dram_tensor("wk", [128, 2, DCH, 3, DCH, 128], fp8, kind="ExternalInput")
    # [p, hl, ci_chunk a, tap t, co]
    wv = nc.dram_tensor("wv", [128, 2, DCH, 3, D], fp8, kind="ExternalInput")
    bq = nc.dram_tensor("bq", [128, DCH], f32, kind="ExternalInput")
    bk = nc.dram_tensor("bk", [128, DCH], f32, kind="ExternalInput")
    bvb = nc.dram_tensor("bvb", [128, D], f32, kind="ExternalInput")
    scl = nc.dram_tensor("scl", [128, 1], f32, kind="ExternalInput")
    out = nc.dram_tensor("out", [SQ // 128, 128, D], f32, kind="ExternalOutput")

    TERMS = ((0, 0), (0, 1), (1, 0))  # (x hl index, w hl index): hh, hl, lh

    with tile.TileContext(nc) as tc:
        with (
            tc.tile_pool(name="persist", bufs=1) as persist,
            tc.tile_pool(name="xin", bufs=3) as xin,
            tc.tile_pool(name="epool", bufs=4) as epool,
            tc.tile_pool(name="vstg", bufs=3) as vstg,
            tc.tile_pool(name="outp", bufs=3) as outp,
            tc.tile_pool(name="tiny", bufs=4) as tiny,
            tc.tile_pool(name="ps_conv", bufs=2, space="PSUM") as ps_conv,
            tc.tile_pool(name="ps_sc", bufs=2, space="PSUM") as ps_sc,
        ):
            # ---- persistent SBUF ----
            wq_s = persist.tile([128, 2, DCH, 3, DCH, 128], fp8, tag="wq_s")
            wk_s = persist.tile([128, 2, DCH, 3, DCH, 128], fp8, tag="wk_s")
            wv_s = persist.tile([128, 2, DCH, 3, D], fp8, tag="wv_s")
            bq_s = persist.tile([128, DCH], f32, tag="bq_s")
            bk_s = persist.tile([128, DCH], f32, tag="bk_s")
            bvb_s = persist.tile([128, D], f32, tag="bvb_s")
            scl_s = persist.tile([128, 1], f32, tag="scl_s")
            qT8 = persist.tile([128, DCH, SQ], fp8, tag="qT8")
            kT8 = persist.tile([128, DCH, SK], fp8, tag="kT8")
            vh_s = persist.tile([128, NKT, VROW], fp8, tag="vh_s")
            vl_s = persist.tile([128, NKT, VROW], fp8e5, tag="vl_s")
            ones8 = persist.tile([128, 2, 128], fp8, tag="ones8")
            sv_sb = persist.tile([128, D + 1], f32, tag="sv_sb")

            # denominator column of v_aug: vh gets 1.0, vl gets 0.0
            nc.vector.memset(vh_s[:, :, D : D + 1], 1.0)
            nc.vector.memset(vl_s[:, :, D : D + 1], 0.0)
            nc.vector.memset(ones8[:], 1.0)

            # q/k T-convs use 2 terms (xh+xl)*wh: the dropped w-residual
            # (~2.7% conv noise) adds in quadrature to the unavoidable fp8
            # cast of q/k, raising the mm1 error term ~1.4x (still in gate)
            QK_TERMS = ((0, 0), (1, 0))

            def conv_T_tile(x_dram, w_s, b_s, out_s, j, tag):
                """T-layout conv col tile j: out_s[:, o, jCT:(j+1)CT] fp8."""
                xt = xin.tile([128, 2, DCH, CT + 16], fp8, tag=tag, name=tag)
                nc.sync.dma_start(
                    xt[:, :, :, 0 : CT + 2],
                    x_dram[:, :, :, j * CT : j * CT + CT + 2],
                )
                last = 3 * len(QK_TERMS) - 1
                for o in range(DCH):
                    ps = ps_conv.tile([128, CT], f32, tag="ps_T", name="ps_T")
                    n = 0
                    for xi, wi in QK_TERMS:
                        for t in range(3):
                            nc.tensor.matmul(
                                ps[:],
                                w_s[:, wi, :, t, o, :],
                                xt[:, xi, :, t : t + CT],
                                start=(n == 0),
                                stop=(n == last),
                                perf_mode=DR,
                            )
                            n += 1
                    nc.vector.tensor_scalar(
                        out=out_s[:, o, j * CT : (j + 1) * CT],
                        in0=ps[:],
                        scalar1=b_s[:, o : o + 1],
                        scalar2=None,
                        op0=ADD,
                    )

            def conv_v_row(xt, j, rr):
                """v conv rowtile -> f32 bias-add (DVE) -> hi/lo fp8 (Pool)."""
                kt = j * (CT // 128) + rr
                ps = ps_conv.tile([128, CT], f32, tag="ps_T", name="ps_T")
                n = 0
                for xi, wi in TERMS:
                    for t in range(3):
                        nc.tensor.matmul(
                            ps[:, 0:D],
                            xt[:, xi, :, t + rr * 128 : t + rr * 128 + 128],
                            wv_s[:, wi, :, t, :],
                            start=(n == 0),
                            stop=(n == 8),
                            perf_mode=DR,
                        )
                        n += 1
                vb = vstg.tile([128, D], f32, tag="vb", name="vb")
                nc.vector.tensor_tensor(
                    out=vb[:], in0=ps[:, 0:D], in1=bvb_s[:], op=ADD
                )
                nc.gpsimd.tensor_copy(out=vh_s[:, kt, 0:D], in_=vb[:])
                nc.gpsimd.tensor_tensor(
                    out=vl_s[:, kt, 0:D], in0=vb[:], in1=vh_s[:, kt, 0:D], op=SUB
                )

            # ---- prefix: weights ride the idle Act HWDGE queue; x tiles
            # own SP from t=0 ----
            nc.scalar.dma_start(wq_s[:], wq[:])
            nc.scalar.dma_start(wk_s[:], wk[:])
            nc.scalar.dma_start(wv_s[:], wv[:])
            nc.scalar.dma_start(bvb_s[:], bvb[:])
            nc.scalar.dma_start(scl_s[:], scl[:])
            nc.sync.dma_start(bq_s[:], bq[:])
            conv_T_tile(xq, wq_s, bq_s, qT8, 0, "xt_q")
            nc.sync.dma_start(bk_s[:], bk[:])
            conv_T_tile(xk, wk_s, bk_s, kT8, 0, "xt_k")

            # ---- filler units interleaved into block 0's mm1 stream ----
            kf = [
                (lambda j=j: conv_T_tile(xk, wk_s, bk_s, kT8, j, "xt_k"))
                for j in range(1, SK // CT)
            ]
            kf.reverse()

            vxts = [None] * (SK // CT)

            def vdma(j):
                xt = xin.tile(
                    [128, 2, DCH, CT + 16], fp8, tag="xt_v", name="xt_v",
                    bufs=SK // CT,
                )
                nc.sync.dma_start(
                    xt[:, :, :, 0 : CT + 2],
                    xv[:, :, :, j * CT : j * CT + CT + 2],
                )
                vxts[j] = xt

            def vrow(r):
                conv_v_row(vxts[r // 4], r // 4, r % 4)

            e_bufs = []

            # mm1 units: 10 kt-triples + 1 kt-pair per block; each triple
            # fills a 3-bank [128,1536] psum tile consumed by ONE activation
            # (fewer, larger exps cut ScalarE per-instr overhead ~5us total)
            def mm1_unit(b, eb, u):
                nk = 3 if u < 10 else 2
                k0 = 3 * u
                ps = ps_sc.tile([128, 1536], f32, tag="ps_sc", name="ps_sc")
                for h in range(nk):
                    kt = k0 + h
                    nc.tensor.matmul(
                        ps[:, h * 512 : h * 512 + 512],
                        kT8[:, :, kt * 128 : (kt + 1) * 128],
                        qT8[:, :, b * QB : (b + 1) * QB],
                        start=True,
                        stop=True,
                        perf_mode=DR,
                    )
                nc.scalar.activation(
                    out=eb[:, k0 : k0 + nk, :],
                    in_=ps[:, 0 : nk * 512],
                    func=EXP,
                    scale=EXP_SCALE,
                )

            # mm2: 4 chunk-parts per qs tile, each 8 DoubleRow kt-pair matmuls
            # parts 0,1 = vh halves; parts 2,3 = vl halves (one psum group)
            states = {}

            def mm2_chunk(b, i):
                eb = e_bufs[b]
                state = states.setdefault(b, {})
                qs, part = divmod(i, 4)
                if part == 0:
                    state["po"] = ps_conv.tile(
                        [128, 512], f32, tag="ps_T", name="ps_T"
                    )
                po = state["po"]
                vt = vh_s if part < 2 else vl_s
                kp0 = 8 * (part % 2)
                for kp in range(kp0, kp0 + 8):
                    nc.tensor.matmul(
                        po[:, 0:DV],
                        eb[:, 2 * kp : 2 * kp + 2, qs * 128 : (qs + 1) * 128],
                        vt[:, 2 * kp : 2 * kp + 2, 0:DV],
                        start=(part == 0 and kp == kp0),
                        stop=(part == 3 and kp == kp0 + 7),
                        perf_mode=DR,
                    )
                if part == 3:
                    tot = outp.tile([128, D + 1], f32, tag="tot", name="tot")
                    nc.vector.tensor_tensor(
                        out=tot[:], in0=po[:, 0 : D + 1], in1=sv_sb[:], op=ADD
                    )
                    rec = tiny.tile([128, 1], f32, tag="rec", name="rec")
                    nc.vector.reciprocal(rec[:], tot[:, D : D + 1])
                    nc.vector.tensor_tensor(
                        out=rec[:], in0=rec[:], in1=scl_s[:], op=MULT
                    )
                    ot = outp.tile([128, D], f32, tag="ot", name="ot")
                    nc.vector.tensor_scalar_mul(ot[:], tot[:, 0:D], rec[:])
                    nc.sync.dma_start(out[b * (QB // 128) + qs, :, :], ot[:])

            def emit_sv():
                sv_ps = ps_conv.tile([128, CT], f32, tag="ps_T", name="ps_T")
                n = 0
                for vt in (vh_s, vl_s):
                    for kp in range(NKT // 2):
                        nc.tensor.matmul(
                            sv_ps[:, 0:DV],
                            ones8[:],
                            vt[:, 2 * kp : 2 * kp + 2, 0:DV],
                            start=(n == 0),
                            stop=(n == NKT - 1),
                            perf_mode=DR,
                        )
                        n += 1
                nc.vector.tensor_copy(out=sv_sb[:], in_=sv_ps[:, 0 : D + 1])

            # ---- attention pipeline ----
            # window 0: mm1(0) against k-conv tiles; v DMAs prefetch;
            # q-conv tile b+1 lands at the end of window b
            eb0 = epool.tile([128, NKT, QB], fp8, tag="e_s", name="e_s")
            e_bufs.append(eb0)
            for u in range(11):
                mm1_unit(0, eb0, u)
                if kf:
                    kf.pop()()
                if u >= 3:
                    vdma(u - 3)
            conv_T_tile(xq, wq_s, bq_s, qT8, 1, "xt_q")

            def U_vrow(r):
                return lambda: vrow(r)

            def U_chunk(b, j):
                return lambda: mm2_chunk(b, j)

            UNITS = {1: [[] for _ in range(11)], 2: [[] for _ in range(11)],
                     3: [[] for _ in range(11)]}
            for r in range(23):          # w1: v rows 0..22
                UNITS[1][(r * 11) // 23].append(U_vrow(r))
            for r in range(23, 32):      # w2: v rows 23..31 on slots 0..3
                UNITS[2][(r - 23) // 3].append(U_vrow(r))
            UNITS[2][4].append(emit_sv)
            for n, (bb, j) in enumerate([(0, j) for j in range(13)]):
                UNITS[2][5 + (n * 6) // 13].append(U_chunk(bb, j))
            W3 = ([(0, 13), (0, 14), (0, 15)] + [(1, j) for j in range(16)]
                  + [(2, j) for j in range(16)])
            for n, (bb, j) in enumerate(W3):
                UNITS[3][(n * 11) // len(W3)].append(U_chunk(bb, j))

            for b in range(1, NBLK):
                eb = epool.tile([128, NKT, QB], fp8, tag="e_s", name="e_s")
                e_bufs.append(eb)
                for u in range(11):
                    mm1_unit(b, eb, u)
                    for un in UNITS[b][u]:
                        un()
                if b < NBLK - 1:
                    conv_T_tile(xq, wq_s, bq_s, qT8, b + 1, "xt_q")
            for bb, j in [(3, j) for j in range(16)]:
                mm2_chunk(bb, j)

    _split_drain_waits(nc)
    return nc


_NC_CACHE = None


def _get_nc():
    global _NC_CACHE
    if _NC_CACHE is None:
        _NC_CACHE = _build_bass()
    return _NC_CACHE


def _split8(a):
    h = a.astype(NP8)
    l = (a - h.astype(np.float32)).astype(NP8)
    return h, l


def _prep_shared(q_w, q_b, k_w, k_b, v_w, v_b, scale):
    def w_T(w):  # [co, ci, 3] -> [p, hl, a, t, o, m] fp8 of 64*w
        arr = np.ascontiguousarray((w * WSCALE).transpose(1, 2, 0)).astype(np.float32)
        arr = arr.reshape(DCH, 128, 3, DCH, 128).transpose(1, 0, 2, 3, 4)
        h, l = _split8(arr)
        return np.ascontiguousarray(np.stack([h, l], axis=1))

    def w_v(w):  # [co, ci, 3] -> [p, hl, a, t, co] fp8 of 64*w
        arr = np.ascontiguousarray((w * WSCALE).transpose(1, 2, 0)).astype(np.float32)
        arr = arr.reshape(DCH, 128, 3, D).transpose(1, 0, 2, 3)
        h, l = _split8(arr)
        return np.ascontiguousarray(np.stack([h, l], axis=1))

    def b_T(b):  # [co] -> [p, o] of 64*b
        return np.ascontiguousarray(
            (np.asarray(b, np.float64) * WSCALE).astype(np.float32).reshape(DCH, 128).T
        )

    return {
        "wq": w_T(q_w),
        "wk": w_T(k_w),
        "wv": w_v(v_w),
        "bq": b_T(q_b),
        "bk": b_T(k_b),
        "bvb": np.ascontiguousarray(
            np.tile((np.asarray(v_b, np.float32) * np.float32(WSCALE)), (128, 1))
        ),
        "scl": np.full((128, 1), float(scale) / WSCALE, np.float32),
    }


def _xT_pad8(x_b):
    """[S, C] -> transposed+halo-padded hi/lo fp8 [128, 2, DCH, S+2]."""
    xt = np.zeros((DIN, x_b.shape[0] + 2), np.float32)
    xt[:, 1:-1] = x_b.T
    h, l = _split8(xt)

    def lay(a):
        return a.reshape(DCH, 128, -1).transpose(1, 0, 2)

    return np.ascontiguousarray(np.stack([lay(h), lay(l)], axis=1))


def kernel(query, key, value, q_w, q_b, k_w, k_b, v_w, v_b, scale):
    _ensure_patched_tables()
    from concourse.bass_utils import run_bass_kernel_spmd

    query = np.asarray(query, np.float32)
    key = np.asarray(key, np.float32)
    value = np.asarray(value, np.float32)

    shared = _prep_shared(
        np.asarray(q_w), np.asarray(q_b), np.asarray(k_w), np.asarray(k_b),
        np.asarray(v_w), np.asarray(v_b), np.asarray(scale),
    )

    in_maps = []
    for c in range(NCORES):
        b, h = c // 2, c % 2
        xq_full = _xT_pad8(query[b])  # [128, 2, DCH, S+2]
        xq_c = np.ascontiguousarray(xq_full[:, :, :, h * SQ : h * SQ + SQ + 2])
        m = dict(shared)
        m["xq"] = xq_c
        m["xk"] = _xT_pad8(key[b])
        m["xv"] = _xT_pad8(value[b])
        in_maps.append(m)

    nc = _get_nc()
    res = run_bass_kernel_spmd(nc, in_maps, core_ids=list(range(NCORES)))

    out_full = np.empty((B, S, D), np.float32)
    for c in range(NCORES):
        b, h = c // 2, c % 2
        out_full[b, h * SQ : (h + 1) * SQ, :] = res.results[c]["out"].reshape(SQ, D)
    return out_full
